# revision 1
# baseline (speedup 1.0000x reference)
"""Locally-connected Conv2d (nn.Conv2dLocal) Trainium2 Bass kernel.

Problem (hardcoded):
  x:      [B=64, C=64, H=32, W=32]  f32
  weight: [OH=32, OW=32, O=64, C=64, KH=3, KW=3] f32
  bias:   [O=64, OH=32, OW=32] f32
  out:    [B=64, O=64, OH=32, OW=32] f32
  out[b,o,oh,ow] = bias[o,oh,ow]
      + sum_{c,kh,kw} x[b,c,oh+kh-1,ow+kw-1] * weight[oh,ow,o,c,kh,kw]

Sharding: 8 cores, core i owns output rows oh in [4i, 4i+4).

Per-core schedule:
  - padded input slab rows r = 0..5 (padded coords), cols iw = 0..33.
  - x strips: strip p = rows (p, p+1), partitions (row, c).
  - per (ow-quarter q, column iw): one weight chunk DMA holding 6 tiles:
      tiles 0..3: kh={0,1} pair for oh=0..3 (K=128 = (kh, c))
      tile  4   : kh=2 for oh=1 (rows 0:64) / oh=0 (rows 64:128)
      tile  5   : kh=2 for oh=2 (rows 0:64) / oh=3 (rows 64:128)
    each tile spans cols (ow asc, o), accumulated into a PSUM bank per
    (q, oh): [64=b, 512=(ow8,o)].
  - bias: rank-1 (ones x bias) bf16 hi/lo matmuls open each bank group.
  - evacuation: ScalarE copy PSUM->SBUF, then DMA out.

Precision MODE:
  "f16x3": x,w split hi+lo fp16; 3 matmuls per logical MM
           (xh*wh + xl*wh + xh*wl) -> ~1e-6 rel err, 1 PE cycle/row.
  "bf16":  single bf16 pass -> ~3e-3 rel err, minimum DMA.
  "fp32":  plain fp32 (4 PE cycles/row).
"""

import numpy as np
import ml_dtypes

B, C, H, W = 64, 64, 32, 32
O, KH, KW = 64, 3, 3
NCORES = 8
RPC = 4              # output rows per core
SLAB = RPC + 2       # padded input rows per core
PW = W + 2           # padded width (34)
NQ = 4               # ow quarters
QW = 8               # ow per quarter
QCOLS = QW + 2       # columns per quarter (10)

MODE = "f16x3"

_cache = {}


def _sched():
    chunks = []
    off = 0
    for q in range(NQ):
        for iw in range(QW * q, QW * q + QCOLS):
            ows = [ow for ow in (iw - 2, iw - 1, iw) if QW * q <= ow < QW * q + QW]
            n = len(ows) * O
            mms = [
                # (oh, strip, p0, psz, tile_idx, tile_p0)
                (0, 0, 0, 128, 0, 0),
                (1, 1, 0, 128, 1, 0),
                (2, 2, 0, 128, 2, 0),
                (3, 3, 0, 128, 3, 0),
                (1, 3, 0, 64, 4, 0),     # kh2: row 3 = strip3 top
                (0, 1, 64, 64, 4, 64),   # kh2: row 2 = strip1 bottom
                (2, 4, 0, 64, 5, 0),     # kh2: row 4 = strip4 top
                (3, 4, 64, 64, 5, 64),   # kh2: row 5 = strip4 bottom
            ]
            chunks.append(dict(q=q, iw=iw, ows=ows, n=n, off=off, mms=mms))
            off += 6 * n
    return chunks, off


def _host_arrays(x, weight, bias):
    """Per-core input dicts, all DMA-contiguous."""
    chunks, total = _sched()
    f16 = ml_dtypes.bfloat16 if MODE == "bf16" else np.float16
    xp = np.pad(x, ((0, 0), (0, 0), (1, 1), (1, 1)))
    in_maps = []
    for i in range(NCORES):
        slab = xp[:, :, RPC * i:RPC * i + SLAB, :]          # [B, C, 6, 34]
        strips = np.stack([
            np.ascontiguousarray(
                slab[:, :, p:p + 2, :].transpose(2, 1, 3, 0).reshape(128, PW * B))
            for p in range(SLAB - 1)
        ])                                                   # [5, 128, 2176] f32
        # per-quarter strip slices (10 columns each), all 5 strips packed
        # into one row of columns: [q, 128, strip * (hi|lo) * 640]
        QC = QCOLS * B                                       # 640
        ns = SLAB - 1
        if MODE == "f16x3":
            xs = np.empty((NQ, 128, ns * 2 * QC), dtype=np.float16)
        elif MODE == "bf16":
            xs = np.empty((NQ, 128, ns * QC), dtype=ml_dtypes.bfloat16)
        else:
            xs = np.empty((NQ, 128, ns * QC), dtype=np.float32)
        for q in range(NQ):
            sl = strips[:, :, QW * q * B:(QW * q + QCOLS) * B]  # [5, 128, 640]
            for s in range(ns):
                if MODE == "f16x3":
                    hi = sl[s].astype(np.float16)
                    lo = (sl[s] - hi.astype(np.float32)).astype(np.float16)
                    xs[q, :, s * 2 * QC:s * 2 * QC + QC] = hi
                    xs[q, :, s * 2 * QC + QC:(s + 1) * 2 * QC] = lo
                else:
                    xs[q, :, s * QC:(s + 1) * QC] = sl[s].astype(xs.dtype)

        w4 = weight[RPC * i:RPC * i + RPC]                   # [4, 32, O, C, 3, 3]
        ws = np.empty((128, total), dtype=np.float32)
        for ch in chunks:
            iw, ows, n, off = ch["iw"], ch["ows"], ch["n"], ch["off"]
            cols = []
            for oh in range(4):                              # tiles 0..3 (kh01)
                blocks = [
                    w4[oh, ow, :, :, 0:2, iw - ow].transpose(2, 1, 0).reshape(128, O)
                    for ow in ows
                ]
                cols.append(np.concatenate(blocks, axis=1))
            for top_oh, bot_oh in ((1, 0), (2, 3)):          # tiles 4, 5 (kh2)
                top = np.concatenate(
                    [w4[top_oh, ow, :, :, 2, iw - ow].T for ow in ows], axis=1)
                bot = np.concatenate(
                    [w4[bot_oh, ow, :, :, 2, iw - ow].T for ow in ows], axis=1)
                cols.append(np.concatenate([top, bot], axis=0))
            ws[:, off:off + 6 * n] = np.concatenate(cols, axis=1)
        if MODE == "f16x3":
            wpk = np.empty((128, 2 * total), dtype=np.float16)
            for ch in chunks:
                n6, off = 6 * ch["n"], ch["off"]
                blk = ws[:, off:off + n6]
                hi = blk.astype(np.float16)
                lo = (blk - hi.astype(np.float32)).astype(np.float16)
                wpk[:, 2 * off:2 * off + n6] = hi
                wpk[:, 2 * off + n6:2 * off + 2 * n6] = lo
        elif MODE == "bf16":
            wpk = ws.astype(ml_dtypes.bfloat16)
        else:
            wpk = ws

        # bias: bf16 hi/lo rows [NQ, 1, 2*2048]
        b4 = bias[:, RPC * i:RPC * i + RPC, :].transpose(1, 2, 0)  # [oh, ow, o]
        bse = np.empty((NQ, 1, 2 * RPC * QW * O), dtype=ml_dtypes.bfloat16)
        for q in range(NQ):
            flat = np.ascontiguousarray(
                b4[:, QW * q:QW * q + QW, :]).reshape(-1)
            hi = flat.astype(ml_dtypes.bfloat16)
            lo = (flat - hi.astype(np.float32)).astype(ml_dtypes.bfloat16)
            bse[q, 0, :flat.size] = hi
            bse[q, 0, flat.size:] = lo
        in_maps.append({"xs": np.ascontiguousarray(xs),
                        "ws": np.ascontiguousarray(wpk), "bse": bse})
    return in_maps


def _build_program():
    from contextlib import ExitStack
    import concourse.bass as bass
    import concourse.bacc as bacc
    import concourse.tile as tile
    from concourse import mybir

    F32 = mybir.dt.float32
    BF16 = mybir.dt.bfloat16
    if MODE == "f16x3":
        WDT, XMUL, WMUL = mybir.dt.float16, 2, 2
    elif MODE == "bf16":
        WDT, XMUL, WMUL = BF16, 1, 1
    else:
        WDT, XMUL, WMUL = F32, 1, 1
    chunks, total = _sched()

    nc = bacc.Bacc("TRN2", target_bir_lowering=False, debug=False,
                   num_devices=NCORES)
    QC = QCOLS * B
    xs_d = nc.dram_tensor("xs", [NQ, 128, (SLAB - 1) * XMUL * QC], WDT,
                          kind="ExternalInput")
    ws_d = nc.dram_tensor("ws", [128, WMUL * total], WDT, kind="ExternalInput")
    bse_d = nc.dram_tensor("bse", [NQ, 1, 2 * RPC * QW * O], BF16,
                           kind="ExternalInput")
    out_d = nc.dram_tensor("out", [B, RPC * W * O], F32, kind="ExternalOutput")

    # stop flag on the last MM per (q, oh) bank group
    laststop = set()
    for q in range(NQ):
        seen = {}
        for ci, ch in enumerate(chunks):
            if ch["q"] != q:
                continue
            for mi, mm in enumerate(ch["mms"]):
                seen.setdefault(mm[0], []).append((ci, mi))
        for oh, lst in seen.items():
            laststop.add(lst[-1])

    with ExitStack() as ctx:
        tc = ctx.enter_context(tile.TileContext(nc))
        xpool = ctx.enter_context(tc.tile_pool(name="xs", bufs=2))
        wpool = ctx.enter_context(tc.tile_pool(name="wt", bufs=4))
        bpool = ctx.enter_context(tc.tile_pool(name="bias", bufs=1))
        opool = ctx.enter_context(tc.tile_pool(name="outs", bufs=2))
        pspool = ctx.enter_context(
            tc.tile_pool(name="ps", bufs=8, space=bass.MemorySpace.PSUM))

        cpool = ctx.enter_context(tc.tile_pool(name="const", bufs=1))
        ones = cpool.tile([1, B], BF16, tag="ones", name="ones")
        nc.gpsimd.memset(ones[:], 1.0)
        NB = 2 * RPC * QW * O  # bias row elems per quarter (hi|lo)
        ball = bpool.tile([1, NQ * NB], BF16, tag="bias", name="bias_all")
        nc.sync.dma_start(ball[:], bse_d.ap().rearrange("q one n -> one (q n)"))

        ws_ap = ws_d.ap()
        out3 = out_d.ap().rearrange("b (oh r) -> b oh r", r=W * O)
        QO = QW * O  # 512, one psum bank
        for q in range(NQ):
            xq = xpool.tile([128, (SLAB - 1) * XMUL * QC], WDT, tag="xq",
                            name=f"xq{q}")
            nc.sync.dma_start(xq[:], xs_d[q])

            def xsl(sp, p0, psz, jl, lo=False):
                base = sp * XMUL * QC + (QC if lo else 0) + jl
                return xq[p0:p0 + psz, base:base + B]

            bt = ball[0:1, q * NB:(q + 1) * NB]
            ps = [pspool.tile([B, QO], F32, tag="psb", name=f"ps{q}_{oh}")
                  for oh in range(RPC)]
            for oh in range(RPC):
                nc.tensor.matmul(ps[oh][:, 0:QO], ones[:],
                                 bt[0:1, oh * QO:(oh + 1) * QO],
                                 start=True, stop=False)
                nc.tensor.matmul(ps[oh][:, 0:QO], ones[:],
                                 bt[0:1, RPC * QO + oh * QO:RPC * QO + (oh + 1) * QO],
                                 start=False, stop=False)
            qchunks = [(ci, ch) for ci, ch in enumerate(chunks) if ch["q"] == q]
            for g in range(0, len(qchunks), 2):              # 2 chunks per DMA
                pair = qchunks[g:g + 2]
                goff = pair[0][1]["off"]
                gcols = sum(6 * ch["n"] for _, ch in pair)
                wt = wpool.tile([128, WMUL * gcols], WDT, tag="wtile",
                                name=f"wt{q}_{g}")
                nc.sync.dma_start(wt[:], ws_ap[:, WMUL * goff:WMUL * (goff + gcols)])
                for ci, ch in pair:
                    iw, ows, n = ch["iw"], ch["ows"], ch["n"]
                    n6 = 6 * n
                    toff = WMUL * (ch["off"] - goff)         # base col in wt
                    c0 = (ows[0] - QW * q) * O
                    jl = (iw - QW * q) * B
                    for mi, mm in enumerate(ch["mms"]):
                        oh, sp, p0, psz, ti, tp0 = mm
                        stop = (ci, mi) in laststop
                        xh = xsl(sp, p0, psz, jl)
                        wh = wt[tp0:tp0 + psz, toff + ti * n:toff + ti * n + n]
                        if MODE == "f16x3":
                            xl = xsl(sp, p0, psz, jl, lo=True)
                            wl = wt[tp0:tp0 + psz,
                                    toff + n6 + ti * n:toff + n6 + ti * n + n]
                            nc.tensor.matmul(ps[oh][:, c0:c0 + n], xh, wh,
                                             start=False, stop=False)
                            nc.tensor.matmul(ps[oh][:, c0:c0 + n], xl, wh,
                                             start=False, stop=False)
                            nc.tensor.matmul(ps[oh][:, c0:c0 + n], xh, wl,
                                             start=False, stop=stop)
                        else:
                            nc.tensor.matmul(ps[oh][:, c0:c0 + n], xh, wh,
                                             start=False, stop=stop)
            ot = opool.tile([B, RPC * QO], F32, tag="ot", name=f"ot{q}")
            for oh in range(RPC):
                nc.scalar.copy(ot[:, oh * QO:(oh + 1) * QO], ps[oh][:])
            nc.sync.dma_start(
                out3[:, :, q * QO:(q + 1) * QO],
                ot[:].rearrange("b (oh r) -> b oh r", r=QO))

    nc.compile()
    return nc


def kernel(x, weight, bias):
    x = np.asarray(x, dtype=np.float32)
    weight = np.asarray(weight, dtype=np.float32)
    bias = np.asarray(bias, dtype=np.float32)

    from concourse.bass_utils import run_bass_kernel_spmd

    if "nc" not in _cache:
        _cache["nc"] = _build_program()
    nc = _cache["nc"]

    in_maps = _host_arrays(x, weight, bias)
    res = run_bass_kernel_spmd(nc, in_maps, list(range(NCORES)))
    out = np.empty((B, O, H, W), dtype=np.float32)
    for i in range(NCORES):
        o_i = res.results[i]["out"].reshape(B, RPC, W, O)   # [b, oh_l, ow, o]
        out[:, :, RPC * i:RPC * i + RPC, :] = o_i.transpose(0, 3, 1, 2)
    return out



# revision 2
# speedup vs baseline: 1.7662x; 1.7662x over previous
"""Locally-connected Conv2d (nn.Conv2dLocal) Trainium2 Bass kernel.

Problem (hardcoded):
  x:      [B=64, C=64, H=32, W=32]  f32
  weight: [OH=32, OW=32, O=64, C=64, KH=3, KW=3] f32
  bias:   [O=64, OH=32, OW=32] f32
  out:    [B=64, O=64, OH=32, OW=32] f32
  out[b,o,oh,ow] = bias[o,oh,ow]
      + sum_{c,kh,kw} x[b,c,oh+kh-1,ow+kw-1] * weight[oh,ow,o,c,kh,kw]

Sharding: 8 cores, core i owns output rows oh in [4i, 4i+4).

Per-core schedule:
  - padded input slab rows r = 0..5 (padded coords), cols iw = 0..33.
  - x strips: strip p = rows (p, p+1), partitions (row, c).
  - per (ow-quarter q, column iw): one weight chunk DMA holding 6 tiles:
      tiles 0..3: kh={0,1} pair for oh=0..3 (K=128 = (kh, c))
      tile  4   : kh=2 for oh=1 (rows 0:64) / oh=0 (rows 64:128)
      tile  5   : kh=2 for oh=2 (rows 0:64) / oh=3 (rows 64:128)
    each tile spans cols (ow asc, o), accumulated into a PSUM bank per
    (q, oh): [64=b, 512=(ow8,o)].
  - bias: rank-1 (ones x bias) bf16 hi/lo matmuls open each bank group.
  - evacuation: ScalarE copy PSUM->SBUF, then DMA out.

Precision MODE:
  "f16x3": x,w split hi+lo fp16; 3 matmuls per logical MM
           (xh*wh + xl*wh + xh*wl) -> ~1e-6 rel err, 1 PE cycle/row.
  "bf16":  single bf16 pass -> ~3e-3 rel err, minimum DMA.
  "fp32":  plain fp32 (4 PE cycles/row).
"""

import numpy as np
import ml_dtypes

B, C, H, W = 64, 64, 32, 32
O, KH, KW = 64, 3, 3
NCORES = 8
RPC = 4              # output rows per core
SLAB = RPC + 2       # padded input rows per core
PW = W + 2           # padded width (34)
NQ = 4               # ow quarters
QW = 8               # ow per quarter
QCOLS = QW + 2       # columns per quarter (10)

MODE = "bf16"

_cache = {}


def _sched():
    chunks = []
    off = 0
    for q in range(NQ):
        for iw in range(QW * q, QW * q + QCOLS):
            ows = [ow for ow in (iw - 2, iw - 1, iw) if QW * q <= ow < QW * q + QW]
            n = len(ows) * O
            mms = [
                # (oh, strip, p0, psz, tile_idx, tile_p0)
                (0, 0, 0, 128, 0, 0),
                (1, 1, 0, 128, 1, 0),
                (2, 2, 0, 128, 2, 0),
                (3, 3, 0, 128, 3, 0),
                (1, 3, 0, 64, 4, 0),     # kh2: row 3 = strip3 top
                (0, 1, 64, 64, 4, 64),   # kh2: row 2 = strip1 bottom
                (2, 4, 0, 64, 5, 0),     # kh2: row 4 = strip4 top
                (3, 4, 64, 64, 5, 64),   # kh2: row 5 = strip4 bottom
            ]
            chunks.append(dict(q=q, iw=iw, ows=ows, n=n, off=off, mms=mms))
            off += 6 * n
    return chunks, off


def _host_arrays(x, weight, bias):
    """Per-core input dicts, all DMA-contiguous."""
    chunks, total = _sched()
    f16 = ml_dtypes.bfloat16 if MODE == "bf16" else np.float16
    xp = np.pad(x, ((0, 0), (0, 0), (1, 1), (1, 1)))
    in_maps = []
    for i in range(NCORES):
        slab = xp[:, :, RPC * i:RPC * i + SLAB, :]          # [B, C, 6, 34]
        strips = np.stack([
            np.ascontiguousarray(
                slab[:, :, p:p + 2, :].transpose(2, 1, 3, 0).reshape(128, PW * B))
            for p in range(SLAB - 1)
        ])                                                   # [5, 128, 2176] f32
        # per-quarter strip slices (10 columns each), all 5 strips packed
        # into one row of columns: [q, 128, strip * (hi|lo) * 640]
        QC = QCOLS * B                                       # 640
        ns = SLAB - 1
        if MODE == "f16x3":
            xs = np.empty((NQ, 128, ns * 2 * QC), dtype=np.float16)
        elif MODE == "bf16":
            xs = np.empty((NQ, 128, ns * QC), dtype=ml_dtypes.bfloat16)
        else:
            xs = np.empty((NQ, 128, ns * QC), dtype=np.float32)
        for q in range(NQ):
            sl = strips[:, :, QW * q * B:(QW * q + QCOLS) * B]  # [5, 128, 640]
            for s in range(ns):
                if MODE == "f16x3":
                    hi = sl[s].astype(np.float16)
                    lo = (sl[s] - hi.astype(np.float32)).astype(np.float16)
                    xs[q, :, s * 2 * QC:s * 2 * QC + QC] = hi
                    xs[q, :, s * 2 * QC + QC:(s + 1) * 2 * QC] = lo
                else:
                    xs[q, :, s * QC:(s + 1) * QC] = sl[s].astype(xs.dtype)

        w4 = weight[RPC * i:RPC * i + RPC]                   # [4, 32, O, C, 3, 3]
        ws = np.empty((128, total), dtype=np.float32)
        for ch in chunks:
            iw, ows, n, off = ch["iw"], ch["ows"], ch["n"], ch["off"]
            cols = []
            for oh in range(4):                              # tiles 0..3 (kh01)
                blocks = [
                    w4[oh, ow, :, :, 0:2, iw - ow].transpose(2, 1, 0).reshape(128, O)
                    for ow in ows
                ]
                cols.append(np.concatenate(blocks, axis=1))
            for top_oh, bot_oh in ((1, 0), (2, 3)):          # tiles 4, 5 (kh2)
                top = np.concatenate(
                    [w4[top_oh, ow, :, :, 2, iw - ow].T for ow in ows], axis=1)
                bot = np.concatenate(
                    [w4[bot_oh, ow, :, :, 2, iw - ow].T for ow in ows], axis=1)
                cols.append(np.concatenate([top, bot], axis=0))
            ws[:, off:off + 6 * n] = np.concatenate(cols, axis=1)
        if MODE == "f16x3":
            wpk = np.empty((128, 2 * total), dtype=np.float16)
            for ch in chunks:
                n6, off = 6 * ch["n"], ch["off"]
                blk = ws[:, off:off + n6]
                hi = blk.astype(np.float16)
                lo = (blk - hi.astype(np.float32)).astype(np.float16)
                wpk[:, 2 * off:2 * off + n6] = hi
                wpk[:, 2 * off + n6:2 * off + 2 * n6] = lo
        elif MODE == "bf16":
            wpk = ws.astype(ml_dtypes.bfloat16)
        else:
            wpk = ws

        # bias: bf16 hi/lo rows [NQ, 1, 2*2048]
        b4 = bias[:, RPC * i:RPC * i + RPC, :].transpose(1, 2, 0)  # [oh, ow, o]
        bse = np.empty((NQ, 1, 2 * RPC * QW * O), dtype=ml_dtypes.bfloat16)
        for q in range(NQ):
            flat = np.ascontiguousarray(
                b4[:, QW * q:QW * q + QW, :]).reshape(-1)
            hi = flat.astype(ml_dtypes.bfloat16)
            lo = (flat - hi.astype(np.float32)).astype(ml_dtypes.bfloat16)
            bse[q, 0, :flat.size] = hi
            bse[q, 0, flat.size:] = lo
        in_maps.append({"xs": np.ascontiguousarray(xs),
                        "ws": np.ascontiguousarray(wpk), "bse": bse})
    return in_maps


def _build_program():
    from contextlib import ExitStack
    import concourse.bass as bass
    import concourse.bacc as bacc
    import concourse.tile as tile
    from concourse import mybir

    F32 = mybir.dt.float32
    BF16 = mybir.dt.bfloat16
    if MODE == "f16x3":
        WDT, XMUL, WMUL = mybir.dt.float16, 2, 2
    elif MODE == "bf16":
        WDT, XMUL, WMUL = BF16, 1, 1
    else:
        WDT, XMUL, WMUL = F32, 1, 1
    chunks, total = _sched()

    nc = bacc.Bacc("TRN2", target_bir_lowering=False, debug=False,
                   num_devices=NCORES)
    QC = QCOLS * B
    xs_d = nc.dram_tensor("xs", [NQ, 128, (SLAB - 1) * XMUL * QC], WDT,
                          kind="ExternalInput")
    ws_d = nc.dram_tensor("ws", [128, WMUL * total], WDT, kind="ExternalInput")
    bse_d = nc.dram_tensor("bse", [NQ, 1, 2 * RPC * QW * O], BF16,
                           kind="ExternalInput")
    out_d = nc.dram_tensor("out", [B, RPC * W * O], F32, kind="ExternalOutput")

    # stop flag on the last MM per (q, oh) bank group
    laststop = set()
    for q in range(NQ):
        seen = {}
        for ci, ch in enumerate(chunks):
            if ch["q"] != q:
                continue
            for mi, mm in enumerate(ch["mms"]):
                seen.setdefault(mm[0], []).append((ci, mi))
        for oh, lst in seen.items():
            laststop.add(lst[-1])

    with ExitStack() as ctx:
        tc = ctx.enter_context(tile.TileContext(nc))
        xpool = ctx.enter_context(tc.tile_pool(name="xs", bufs=2))
        wpool = ctx.enter_context(tc.tile_pool(name="wt", bufs=4))
        bpool = ctx.enter_context(tc.tile_pool(name="bias", bufs=1))
        opool = ctx.enter_context(tc.tile_pool(name="outs", bufs=2))
        pspool = ctx.enter_context(
            tc.tile_pool(name="ps", bufs=8, space=bass.MemorySpace.PSUM))

        cpool = ctx.enter_context(tc.tile_pool(name="const", bufs=1))
        ones = cpool.tile([1, B], BF16, tag="ones", name="ones")
        nc.gpsimd.memset(ones[:], 1.0)
        NB = 2 * RPC * QW * O  # bias row elems per quarter (hi|lo)
        ball = bpool.tile([1, NQ * NB], BF16, tag="bias", name="bias_all")
        nc.sync.dma_start(ball[:], bse_d.ap().rearrange("q one n -> one (q n)"))

        ws_ap = ws_d.ap()
        out3 = out_d.ap().rearrange("b (oh r) -> b oh r", r=W * O)
        QO = QW * O  # 512, one psum bank
        for q in range(NQ):
            xq = xpool.tile([128, (SLAB - 1) * XMUL * QC], WDT, tag="xq",
                            name=f"xq{q}")
            nc.sync.dma_start(xq[:], xs_d[q])

            def xsl(sp, p0, psz, jl, lo=False):
                base = sp * XMUL * QC + (QC if lo else 0) + jl
                return xq[p0:p0 + psz, base:base + B]

            bt = ball[0:1, q * NB:(q + 1) * NB]
            ps = [pspool.tile([B, QO], F32, tag="psb", name=f"ps{q}_{oh}")
                  for oh in range(RPC)]
            for oh in range(RPC):
                nc.tensor.matmul(ps[oh][:, 0:QO], ones[:],
                                 bt[0:1, oh * QO:(oh + 1) * QO],
                                 start=True, stop=False)
                nc.tensor.matmul(ps[oh][:, 0:QO], ones[:],
                                 bt[0:1, RPC * QO + oh * QO:RPC * QO + (oh + 1) * QO],
                                 start=False, stop=False)
            qchunks = [(ci, ch) for ci, ch in enumerate(chunks) if ch["q"] == q]
            for g in range(0, len(qchunks), 2):              # 2 chunks per DMA
                pair = qchunks[g:g + 2]
                goff = pair[0][1]["off"]
                gcols = sum(6 * ch["n"] for _, ch in pair)
                wt = wpool.tile([128, WMUL * gcols], WDT, tag="wtile",
                                name=f"wt{q}_{g}")
                nc.sync.dma_start(wt[:], ws_ap[:, WMUL * goff:WMUL * (goff + gcols)])
                for ci, ch in pair:
                    iw, ows, n = ch["iw"], ch["ows"], ch["n"]
                    n6 = 6 * n
                    toff = WMUL * (ch["off"] - goff)         # base col in wt
                    c0 = (ows[0] - QW * q) * O
                    jl = (iw - QW * q) * B
                    for mi, mm in enumerate(ch["mms"]):
                        oh, sp, p0, psz, ti, tp0 = mm
                        stop = (ci, mi) in laststop
                        xh = xsl(sp, p0, psz, jl)
                        wh = wt[tp0:tp0 + psz, toff + ti * n:toff + ti * n + n]
                        if MODE == "f16x3":
                            xl = xsl(sp, p0, psz, jl, lo=True)
                            wl = wt[tp0:tp0 + psz,
                                    toff + n6 + ti * n:toff + n6 + ti * n + n]
                            nc.tensor.matmul(ps[oh][:, c0:c0 + n], xh, wh,
                                             start=False, stop=False)
                            nc.tensor.matmul(ps[oh][:, c0:c0 + n], xl, wh,
                                             start=False, stop=False)
                            nc.tensor.matmul(ps[oh][:, c0:c0 + n], xh, wl,
                                             start=False, stop=stop)
                        else:
                            nc.tensor.matmul(ps[oh][:, c0:c0 + n], xh, wh,
                                             start=False, stop=stop)
            ot = opool.tile([B, RPC * QO], F32, tag="ot", name=f"ot{q}")
            for oh in range(RPC):
                nc.scalar.copy(ot[:, oh * QO:(oh + 1) * QO], ps[oh][:])
            nc.sync.dma_start(
                out3[:, :, q * QO:(q + 1) * QO],
                ot[:].rearrange("b (oh r) -> b oh r", r=QO))

    nc.compile()
    return nc


def kernel(x, weight, bias):
    x = np.asarray(x, dtype=np.float32)
    weight = np.asarray(weight, dtype=np.float32)
    bias = np.asarray(bias, dtype=np.float32)

    from concourse.bass_utils import run_bass_kernel_spmd

    if "nc" not in _cache:
        _cache["nc"] = _build_program()
    nc = _cache["nc"]

    in_maps = _host_arrays(x, weight, bias)
    res = run_bass_kernel_spmd(nc, in_maps, list(range(NCORES)))
    out = np.empty((B, O, H, W), dtype=np.float32)
    for i in range(NCORES):
        o_i = res.results[i]["out"].reshape(B, RPC, W, O)   # [b, oh_l, ow, o]
        out[:, :, RPC * i:RPC * i + RPC, :] = o_i.transpose(0, 3, 1, 2)
    return out



# revision 3
# speedup vs baseline: 1.9865x; 1.1247x over previous
"""Locally-connected Conv2d (nn.Conv2dLocal) Trainium2 Bass kernel.

Problem (hardcoded):
  x:      [B=64, C=64, H=32, W=32]  f32
  weight: [OH=32, OW=32, O=64, C=64, KH=3, KW=3] f32
  bias:   [O=64, OH=32, OW=32] f32
  out:    [B=64, O=64, OH=32, OW=32] f32
  out[b,o,oh,ow] = bias[o,oh,ow]
      + sum_{c,kh,kw} x[b,c,oh+kh-1,ow+kw-1] * weight[oh,ow,o,c,kh,kw]

Sharding: 8 cores, core i owns output rows oh in [4i, 4i+4).

DMA-minimal fp16 design (~12.2 MB/core, DMA-bound at 360 B/ns):
  - x slab: padded rows 0..5 stored ONCE as 3 even "double rows"
    dr = rows (2dr, 2dr+1), partitions (row parity, c), cols (dr, iw, b).
    [128, 3*34*64] fp16 = 1.6 MB, single DMA, resident all kernel.
  - per output row oh the 3 kh taps split as one full-K pair + one
    half-K single against the even-pair layout:
      oh even: pair (kh0,kh1) = dr oh/2 full;  single kh2 = dr(oh/2+1) top
      oh odd:  single kh0 = dr (oh-1)/2 bottom; pair (kh1,kh2) = dr(oh+1)/2 full
  - weights streamed once, fp16 [128, 36864] = 9.4 MB; per (q, iw)
    chunk 6 tiles: t0..t3 = per-oh kh pairs, t4/t5 = packed singles.
  - bias: one fp16 rank-1 matmul opens each (q, oh) PSUM bank.
  - out: PSUM -> fp16 SBUF (ScalarE cast) -> DMA, 1.05 MB.
  Rel err ~4e-4 (fp16 rounding of x, w, out; f32 accumulation).
"""

import numpy as np

B, C, H, W = 64, 64, 32, 32
O, KH, KW = 64, 3, 3
NCORES = 8
RPC = 4              # output rows per core
SLAB = RPC + 2       # padded input rows per core
PW = W + 2           # padded width (34)
NDR = SLAB // 2      # even double-rows per slab (3)
NQ = 4               # ow quarters
QW = 8               # ow per quarter
QCOLS = QW + 2       # iw columns per quarter (10)
QO = QW * O          # psum bank cols per (q, oh) (512)

_cache = {}


def _sched():
    chunks = []
    off = 0
    for q in range(NQ):
        for iw in range(QW * q, QW * q + QCOLS):
            ows = [ow for ow in (iw - 2, iw - 1, iw) if QW * q <= ow < QW * q + QW]
            n = len(ows) * O
            # (oh, dr, p0, psz, tile_idx, tile_p0)
            mms = [
                (0, 0, 0, 128, 0, 0),    # oh0 kh(0,1) = dr0 full
                (1, 1, 0, 128, 1, 0),    # oh1 kh(1,2) = dr1 full
                (2, 1, 0, 128, 2, 0),    # oh2 kh(0,1) = dr1 full
                (3, 2, 0, 128, 3, 0),    # oh3 kh(1,2) = dr2 full
                (0, 1, 0, 64, 4, 0),     # oh0 kh2 = row2 = dr1 top
                (1, 0, 64, 64, 4, 64),   # oh1 kh0 = row1 = dr0 bottom
                (2, 2, 0, 64, 5, 0),     # oh2 kh2 = row4 = dr2 top
                (3, 1, 64, 64, 5, 64),   # oh3 kh0 = row3 = dr1 bottom
            ]
            chunks.append(dict(q=q, iw=iw, ows=ows, n=n, off=off, mms=mms))
            off += 6 * n
    return chunks, off


def _host_arrays(x, weight, bias):
    """Per-core input dicts, all DMA-contiguous, fp16."""
    chunks, total = _sched()
    xp = np.pad(x, ((0, 0), (0, 0), (1, 1), (1, 1)))
    in_maps = []
    for i in range(NCORES):
        slab = xp[:, :, RPC * i:RPC * i + SLAB, :]          # [B, C, 6, 34]
        xs = np.empty((128, NDR * PW * B), dtype=np.float16)
        for dr in range(NDR):
            pair = slab[:, :, 2 * dr:2 * dr + 2, :]          # [B, C, 2, 34]
            xs[:, dr * PW * B:(dr + 1) * PW * B] = (
                pair.transpose(2, 1, 3, 0).reshape(128, PW * B))

        w4 = weight[RPC * i:RPC * i + RPC]                   # [4, 32, O, C, 3, 3]
        ws = np.empty((128, total), dtype=np.float16)
        for ch in chunks:
            iw, ows, n, off = ch["iw"], ch["ows"], ch["n"], ch["off"]
            cols = []
            for oh, k0 in ((0, 0), (1, 1), (2, 0), (3, 1)):  # t0..t3 kh pairs
                blocks = [
                    w4[oh, ow, :, :, k0:k0 + 2, iw - ow].transpose(2, 1, 0)
                    .reshape(128, O)
                    for ow in ows
                ]
                cols.append(np.concatenate(blocks, axis=1))
            for top_oh, bot_oh in ((0, 1), (2, 3)):          # t4, t5 singles
                top = np.concatenate(
                    [w4[top_oh, ow, :, :, 2, iw - ow].T for ow in ows], axis=1)
                bot = np.concatenate(
                    [w4[bot_oh, ow, :, :, 0, iw - ow].T for ow in ows], axis=1)
                cols.append(np.concatenate([top, bot], axis=0))
            ws[:, off:off + 6 * n] = np.concatenate(cols, axis=1)

        # bias rows: [1, (q, oh, ow, o)]
        b4 = bias[:, RPC * i:RPC * i + RPC, :].transpose(1, 2, 0)  # [oh, ow, o]
        bse = np.empty((1, NQ * RPC * QO), dtype=np.float16)
        for q in range(NQ):
            bse[0, q * RPC * QO:(q + 1) * RPC * QO] = (
                b4[:, QW * q:QW * q + QW, :].reshape(-1))
        in_maps.append({"xs": np.ascontiguousarray(xs),
                        "ws": np.ascontiguousarray(ws), "bse": bse})
    return in_maps


def _build_program():
    from contextlib import ExitStack
    import concourse.bass as bass
    import concourse.bacc as bacc
    import concourse.tile as tile
    from concourse import mybir

    F16 = mybir.dt.float16
    F32 = mybir.dt.float32
    chunks, total = _sched()

    nc = bacc.Bacc("TRN2", target_bir_lowering=False, debug=False,
                   num_devices=NCORES)
    xs_d = nc.dram_tensor("xs", [128, NDR * PW * B], F16, kind="ExternalInput")
    ws_d = nc.dram_tensor("ws", [128, total], F16, kind="ExternalInput")
    bse_d = nc.dram_tensor("bse", [1, NQ * RPC * QO], F16, kind="ExternalInput")
    out_d = nc.dram_tensor("out", [B, RPC * W * O], F16, kind="ExternalOutput")

    # stop flag on the last MM per (q, oh) bank group
    laststop = set()
    for q in range(NQ):
        seen = {}
        for ci, ch in enumerate(chunks):
            if ch["q"] != q:
                continue
            for mi, mm in enumerate(ch["mms"]):
                seen.setdefault(mm[0], []).append((ci, mi))
        for oh, lst in seen.items():
            laststop.add(lst[-1])

    with ExitStack() as ctx:
        tc = ctx.enter_context(tile.TileContext(nc))
        xpool = ctx.enter_context(tc.tile_pool(name="xs", bufs=1))
        wpool = ctx.enter_context(tc.tile_pool(name="wt", bufs=4))
        bpool = ctx.enter_context(tc.tile_pool(name="bias", bufs=1))
        opool = ctx.enter_context(tc.tile_pool(name="outs", bufs=2))
        pspool = ctx.enter_context(
            tc.tile_pool(name="ps", bufs=8, space=bass.MemorySpace.PSUM))

        cpool = ctx.enter_context(tc.tile_pool(name="const", bufs=1))
        ones = cpool.tile([1, B], F16, tag="ones", name="ones")
        nc.gpsimd.memset(ones[:], 1.0)
        NB = RPC * QO  # bias elems per quarter (2048)
        ball = bpool.tile([1, NQ * NB], F16, tag="bias", name="bias_all")
        nc.sync.dma_start(ball[:], bse_d.ap())

        xq = xpool.tile([128, NDR * PW * B], F16, tag="xq", name="xq")
        nc.sync.dma_start(xq[:], xs_d.ap())

        def xsl(dr, p0, psz, iw):
            base = (dr * PW + iw) * B
            return xq[p0:p0 + psz, base:base + B]

        ws_ap = ws_d.ap()
        out3 = out_d.ap().rearrange("b (oh r) -> b oh r", r=W * O)
        for q in range(NQ):
            bt = ball[0:1, q * NB:(q + 1) * NB]
            ps = [pspool.tile([B, QO], F32, tag="psb", name=f"ps{q}_{oh}")
                  for oh in range(RPC)]
            for oh in range(RPC):
                nc.tensor.matmul(ps[oh][:, 0:QO], ones[:],
                                 bt[0:1, oh * QO:(oh + 1) * QO],
                                 start=True, stop=False)
            qchunks = [(ci, ch) for ci, ch in enumerate(chunks) if ch["q"] == q]
            for g in range(0, len(qchunks), 2):              # 2 chunks per DMA
                pair = qchunks[g:g + 2]
                goff = pair[0][1]["off"]
                gcols = sum(6 * ch["n"] for _, ch in pair)
                wt = wpool.tile([128, gcols], F16, tag="wtile",
                                name=f"wt{q}_{g}")
                nc.sync.dma_start(wt[:], ws_ap[:, goff:goff + gcols])
                for ci, ch in pair:
                    iw, ows, n = ch["iw"], ch["ows"], ch["n"]
                    toff = ch["off"] - goff                  # base col in wt
                    c0 = (ows[0] - QW * q) * O
                    for mi, mm in enumerate(ch["mms"]):
                        oh, dr, p0, psz, ti, tp0 = mm
                        stop = (ci, mi) in laststop
                        xh = xsl(dr, p0, psz, iw)
                        wh = wt[tp0:tp0 + psz, toff + ti * n:toff + ti * n + n]
                        nc.tensor.matmul(ps[oh][:, c0:c0 + n], xh, wh,
                                         start=False, stop=stop)
            ot = opool.tile([B, RPC * QO], F16, tag="ot", name=f"ot{q}")
            for oh in range(RPC):
                nc.scalar.copy(ot[:, oh * QO:(oh + 1) * QO], ps[oh][:])
            nc.sync.dma_start(
                out3[:, :, q * QO:(q + 1) * QO],
                ot[:].rearrange("b (oh r) -> b oh r", r=QO))

    nc.compile()
    return nc


def kernel(x, weight, bias):
    x = np.asarray(x, dtype=np.float32)
    weight = np.asarray(weight, dtype=np.float32)
    bias = np.asarray(bias, dtype=np.float32)

    from concourse.bass_utils import run_bass_kernel_spmd

    if "nc" not in _cache:
        _cache["nc"] = _build_program()
    nc = _cache["nc"]

    in_maps = _host_arrays(x, weight, bias)
    res = run_bass_kernel_spmd(nc, in_maps, list(range(NCORES)))
    out = np.empty((B, O, H, W), dtype=np.float32)
    for i in range(NCORES):
        o_i = res.results[i]["out"].astype(np.float32)
        o_i = o_i.reshape(B, RPC, W, O)                     # [b, oh_l, ow, o]
        out[:, :, RPC * i:RPC * i + RPC, :] = o_i.transpose(0, 3, 1, 2)
    return out


# revision 4
# speedup vs baseline: 2.1137x; 1.0641x over previous
"""Locally-connected Conv2d (nn.Conv2dLocal) Trainium2 Bass kernel.

Problem (hardcoded):
  x:      [B=64, C=64, H=32, W=32]  f32
  weight: [OH=32, OW=32, O=64, C=64, KH=3, KW=3] f32
  bias:   [O=64, OH=32, OW=32] f32
  out:    [B=64, O=64, OH=32, OW=32] f32
  out[b,o,oh,ow] = bias[o,oh,ow]
      + sum_{c,kh,kw} x[b,c,oh+kh-1,ow+kw-1] * weight[oh,ow,o,c,kh,kw]

Sharding: 8 cores, core i owns output rows oh in [4i, 4i+4).

DMA-minimal fp16 design (~12.2 MB/core, DMA-bound at 360 B/ns):
  - x slab: padded rows 0..5 stored ONCE as 3 even "double rows"
    dr = rows (2dr, 2dr+1), partitions (row parity, c), cols
    (iw, dr, b) iw-major + a leading ones block for the bias matmul.
    [128, 64 + 34*3*64] fp16 = 1.6 MB, DMA'd in 4 iw-range pieces
    interleaved with the weight stream so compute starts early.
  - per output row oh the 3 kh taps split as one full-K pair + one
    half-K single against the even-pair layout:
      oh even: pair (kh0,kh1) = dr oh/2 full;  single kh2 = dr(oh/2+1) top
      oh odd:  single kh0 = dr (oh-1)/2 bottom; pair (kh1,kh2) = dr(oh+1)/2 full
  - weights streamed once, fp16 [128, 36864] = 9.4 MB; per (q, iw)
    chunk 6 tiles: t0..t3 = per-oh kh pairs, t4/t5 = packed singles.
  - bias: one fp16 rank-1 matmul opens each (q, oh) PSUM bank.
  - per quarter one 4-bank PSUM tile [64, 2048]; single wide ScalarE
    cast to fp16 SBUF; out DMA issued from the Activation queue
    (no cross-engine semaphore hop). out = 1.05 MB.
  Rel err ~4e-4 (fp16 rounding of x, w, out; f32 accumulation).
"""

import numpy as np

B, C, H, W = 64, 64, 32, 32
O, KH, KW = 64, 3, 3
NCORES = 8
RPC = 4              # output rows per core
SLAB = RPC + 2       # padded input rows per core
PW = W + 2           # padded width (34)
NDR = SLAB // 2      # even double-rows per slab (3)
NQ = 4               # ow quarters
QW = 8               # ow per quarter
QCOLS = QW + 2       # iw columns per quarter (10)
QO = QW * O          # psum bank cols per (q, oh) (512)
XCOLS = B + PW * NDR * B          # ones block + x cols
XPIECE = (10, 18, 26, PW)         # iw piece upper bounds per quarter

_cache = {}


def _sched():
    chunks = []
    off = 0
    for q in range(NQ):
        for iw in range(QW * q, QW * q + QCOLS):
            ows = [ow for ow in (iw - 2, iw - 1, iw) if QW * q <= ow < QW * q + QW]
            n = len(ows) * O
            # (oh, dr, p0, psz, tile_idx, tile_p0)
            mms = [
                (0, 0, 0, 128, 0, 0),    # oh0 kh(0,1) = dr0 full
                (1, 1, 0, 128, 1, 0),    # oh1 kh(1,2) = dr1 full
                (2, 1, 0, 128, 2, 0),    # oh2 kh(0,1) = dr1 full
                (3, 2, 0, 128, 3, 0),    # oh3 kh(1,2) = dr2 full
                (0, 1, 0, 64, 4, 0),     # oh0 kh2 = row2 = dr1 top
                (1, 0, 64, 64, 4, 64),   # oh1 kh0 = row1 = dr0 bottom
                (2, 2, 0, 64, 5, 0),     # oh2 kh2 = row4 = dr2 top
                (3, 1, 64, 64, 5, 64),   # oh3 kh0 = row3 = dr1 bottom
            ]
            chunks.append(dict(q=q, iw=iw, ows=ows, n=n, off=off, mms=mms))
            off += 6 * n
    return chunks, off


def _host_arrays(x, weight, bias):
    """Per-core input dicts, all DMA-contiguous, fp16."""
    chunks, total = _sched()
    xp = np.pad(x, ((0, 0), (0, 0), (1, 1), (1, 1)))
    in_maps = []
    for i in range(NCORES):
        slab = xp[:, :, RPC * i:RPC * i + SLAB, :]          # [B, C, 6, 34]
        xs = np.empty((128, XCOLS), dtype=np.float16)
        xs[:, 0:B] = 1.0                                     # ones block
        xr = xs[:, B:].reshape(128, PW, NDR, B)
        for dr in range(NDR):
            pair = slab[:, :, 2 * dr:2 * dr + 2, :]          # [B, C, 2, 34]
            xr[:, :, dr, :] = pair.transpose(2, 1, 3, 0).reshape(128, PW, B)

        w4 = weight[RPC * i:RPC * i + RPC]                   # [4, 32, O, C, 3, 3]
        ws = np.empty((128, total), dtype=np.float16)
        for ch in chunks:
            iw, ows, n, off = ch["iw"], ch["ows"], ch["n"], ch["off"]
            cols = []
            for oh, k0 in ((0, 0), (1, 1), (2, 0), (3, 1)):  # t0..t3 kh pairs
                blocks = [
                    w4[oh, ow, :, :, k0:k0 + 2, iw - ow].transpose(2, 1, 0)
                    .reshape(128, O)
                    for ow in ows
                ]
                cols.append(np.concatenate(blocks, axis=1))
            for top_oh, bot_oh in ((0, 1), (2, 3)):          # t4, t5 singles
                top = np.concatenate(
                    [w4[top_oh, ow, :, :, 2, iw - ow].T for ow in ows], axis=1)
                bot = np.concatenate(
                    [w4[bot_oh, ow, :, :, 0, iw - ow].T for ow in ows], axis=1)
                cols.append(np.concatenate([top, bot], axis=0))
            ws[:, off:off + 6 * n] = np.concatenate(cols, axis=1)

        # bias rows: [1, (q, oh, ow, o)]
        b4 = bias[:, RPC * i:RPC * i + RPC, :].transpose(1, 2, 0)  # [oh, ow, o]
        bse = np.empty((1, NQ * RPC * QO), dtype=np.float16)
        for q in range(NQ):
            bse[0, q * RPC * QO:(q + 1) * RPC * QO] = (
                b4[:, QW * q:QW * q + QW, :].reshape(-1))
        in_maps.append({"xs": np.ascontiguousarray(xs),
                        "ws": np.ascontiguousarray(ws), "bse": bse})
    return in_maps


def _build_program():
    from contextlib import ExitStack
    import concourse.bass as bass
    import concourse.bacc as bacc
    import concourse.tile as tile
    from concourse import mybir

    F16 = mybir.dt.float16
    F32 = mybir.dt.float32
    chunks, total = _sched()

    nc = bacc.Bacc("TRN2", target_bir_lowering=False, debug=False,
                   num_devices=NCORES)
    xs_d = nc.dram_tensor("xs", [128, XCOLS], F16, kind="ExternalInput")
    ws_d = nc.dram_tensor("ws", [128, total], F16, kind="ExternalInput")
    bse_d = nc.dram_tensor("bse", [1, NQ * RPC * QO], F16, kind="ExternalInput")
    out_d = nc.dram_tensor("out", [B, RPC * W * O], F16, kind="ExternalOutput")

    # stop flag on the last MM per (q, oh) bank group
    laststop = set()
    for q in range(NQ):
        seen = {}
        for ci, ch in enumerate(chunks):
            if ch["q"] != q:
                continue
            for mi, mm in enumerate(ch["mms"]):
                seen.setdefault(mm[0], []).append((ci, mi))
        for oh, lst in seen.items():
            laststop.add(lst[-1])

    with ExitStack() as ctx:
        tc = ctx.enter_context(tile.TileContext(nc))
        xpool = ctx.enter_context(tc.tile_pool(name="xs", bufs=1))
        wpool = ctx.enter_context(tc.tile_pool(name="wt", bufs=6))
        bpool = ctx.enter_context(tc.tile_pool(name="bias", bufs=1))
        opool = ctx.enter_context(tc.tile_pool(name="outs", bufs=2))
        pspool = ctx.enter_context(
            tc.tile_pool(name="ps", bufs=2, space=bass.MemorySpace.PSUM))

        NB = RPC * QO  # bias elems per quarter (2048)
        ball = bpool.tile([1, NQ * NB], F16, tag="bias", name="bias_all")
        nc.sync.dma_start(ball[:], bse_d.ap())

        xq = xpool.tile([128, XCOLS], F16, tag="xq", name="xq")
        ones = xq[0:1, 0:B]

        def xsl(dr, p0, psz, iw):
            base = B + (iw * NDR + dr) * B
            return xq[p0:p0 + psz, base:base + B]

        ws_ap = ws_d.ap()
        xs_ap = xs_d.ap()
        out3 = out_d.ap().rearrange("b (oh r) -> b oh r", r=W * O)
        xlo = 0
        for q in range(NQ):
            # x piece for this quarter's new iw range (piece 0 incl. ones)
            xhi = B + XPIECE[q] * NDR * B
            nc.sync.dma_start(xq[:, xlo:xhi], xs_ap[:, xlo:xhi])
            xlo = xhi

            bt = ball[0:1, q * NB:(q + 1) * NB]
            ps = pspool.tile([B, RPC * QO], F32, tag="psb", name=f"ps{q}")
            for oh in range(RPC):
                nc.tensor.matmul(ps[:, oh * QO:(oh + 1) * QO], ones,
                                 bt[0:1, oh * QO:(oh + 1) * QO],
                                 start=True, stop=False)
            qchunks = [(ci, ch) for ci, ch in enumerate(chunks) if ch["q"] == q]
            for g in range(0, len(qchunks), 2):              # 2 chunks per DMA
                pair = qchunks[g:g + 2]
                goff = pair[0][1]["off"]
                gcols = sum(6 * ch["n"] for _, ch in pair)
                wt = wpool.tile([128, gcols], F16, tag="wtile",
                                name=f"wt{q}_{g}")
                nc.sync.dma_start(wt[:], ws_ap[:, goff:goff + gcols])
                for ci, ch in pair:
                    iw, ows, n = ch["iw"], ch["ows"], ch["n"]
                    toff = ch["off"] - goff                  # base col in wt
                    c0 = (ows[0] - QW * q) * O
                    for mi, mm in enumerate(ch["mms"]):
                        oh, dr, p0, psz, ti, tp0 = mm
                        stop = (ci, mi) in laststop
                        xh = xsl(dr, p0, psz, iw)
                        wh = wt[tp0:tp0 + psz, toff + ti * n:toff + ti * n + n]
                        nc.tensor.matmul(ps[:, oh * QO + c0:oh * QO + c0 + n],
                                         xh, wh, start=False, stop=stop)
            ot = opool.tile([B, RPC * QO], F16, tag="ot", name=f"ot{q}")
            nc.scalar.copy(ot[:], ps[:])
            nc.scalar.dma_start(
                out3[:, :, q * QO:(q + 1) * QO],
                ot[:].rearrange("b (oh r) -> b oh r", r=QO))

    nc.compile()
    return nc


def kernel(x, weight, bias):
    x = np.asarray(x, dtype=np.float32)
    weight = np.asarray(weight, dtype=np.float32)
    bias = np.asarray(bias, dtype=np.float32)

    from concourse.bass_utils import run_bass_kernel_spmd

    if "nc" not in _cache:
        _cache["nc"] = _build_program()
    nc = _cache["nc"]

    in_maps = _host_arrays(x, weight, bias)
    res = run_bass_kernel_spmd(nc, in_maps, list(range(NCORES)))
    out = np.empty((B, O, H, W), dtype=np.float32)
    for i in range(NCORES):
        o_i = res.results[i]["out"].astype(np.float32)
        o_i = o_i.reshape(B, RPC, W, O)                     # [b, oh_l, ow, o]
        out[:, :, RPC * i:RPC * i + RPC, :] = o_i.transpose(0, 3, 1, 2)
    return out


# revision 6
# speedup vs baseline: 2.1160x; 1.0011x over previous
"""Locally-connected Conv2d (nn.Conv2dLocal) Trainium2 Bass kernel.

Problem (hardcoded):
  x:      [B=64, C=64, H=32, W=32]  f32
  weight: [OH=32, OW=32, O=64, C=64, KH=3, KW=3] f32
  bias:   [O=64, OH=32, OW=32] f32
  out:    [B=64, O=64, OH=32, OW=32] f32
  out[b,o,oh,ow] = bias[o,oh,ow]
      + sum_{c,kh,kw} x[b,c,oh+kh-1,ow+kw-1] * weight[oh,ow,o,c,kh,kw]

Sharding: 8 cores, core i owns output rows oh in [4i, 4i+4).

DMA-minimal fp16 design (~12.2 MB/core, DMA-bound at 360 B/ns):
  - x slab: padded rows 0..5 stored ONCE as 3 even "double rows"
    dr = rows (2dr, 2dr+1), partitions (row parity, c), cols
    (iw, dr, b) iw-major + a leading ones block for the bias matmul.
    [128, 64 + 34*3*64] fp16 = 1.6 MB, DMA'd in 4 iw-range pieces
    interleaved with the weight stream so compute starts early.
  - per output row oh the 3 kh taps split as one full-K pair + one
    half-K single against the even-pair layout:
      oh even: pair (kh0,kh1) = dr oh/2 full;  single kh2 = dr(oh/2+1) top
      oh odd:  single kh0 = dr (oh-1)/2 bottom; pair (kh1,kh2) = dr(oh+1)/2 full
  - weights streamed once, fp16 [128, 36864] = 9.4 MB; per (q, iw)
    chunk 6 tiles: t0..t3 = per-oh kh pairs, t4/t5 = packed singles.
  - bias: one fp16 rank-1 matmul opens each (q, oh) PSUM bank.
  - per quarter one 4-bank PSUM tile [64, 2048]; single wide ScalarE
    cast to fp16 SBUF; out DMA issued from the Activation queue
    (no cross-engine semaphore hop). out = 1.05 MB.
  Rel err ~4e-4 (fp16 rounding of x, w, out; f32 accumulation).
"""

import numpy as np

B, C, H, W = 64, 64, 32, 32
O, KH, KW = 64, 3, 3
NCORES = 8
RPC = 4              # output rows per core
SLAB = RPC + 2       # padded input rows per core
PW = W + 2           # padded width (34)
NDR = SLAB // 2      # even double-rows per slab (3)
NQ = 4               # ow quarters
QW = 8               # ow per quarter
QCOLS = QW + 2       # iw columns per quarter (10)
QO = QW * O          # psum bank cols per (q, oh) (512)
XCOLS = B + PW * NDR * B          # ones block + x cols
XPIECE = (10, 18, 26, PW)         # iw piece upper bounds per quarter

_cache = {}


def _sched():
    chunks = []
    off = 0
    for q in range(NQ):
        for iw in range(QW * q, QW * q + QCOLS):
            ows = [ow for ow in (iw - 2, iw - 1, iw) if QW * q <= ow < QW * q + QW]
            n = len(ows) * O
            # (oh, dr, p0, psz, tile_idx, tile_p0)
            mms = [
                (0, 0, 0, 128, 0, 0),    # oh0 kh(0,1) = dr0 full
                (1, 1, 0, 128, 1, 0),    # oh1 kh(1,2) = dr1 full
                (2, 1, 0, 128, 2, 0),    # oh2 kh(0,1) = dr1 full
                (3, 2, 0, 128, 3, 0),    # oh3 kh(1,2) = dr2 full
                (0, 1, 0, 64, 4, 0),     # oh0 kh2 = row2 = dr1 top
                (1, 0, 64, 64, 4, 64),   # oh1 kh0 = row1 = dr0 bottom
                (2, 2, 0, 64, 5, 0),     # oh2 kh2 = row4 = dr2 top
                (3, 1, 64, 64, 5, 64),   # oh3 kh0 = row3 = dr1 bottom
            ]
            chunks.append(dict(q=q, iw=iw, ows=ows, n=n, off=off, mms=mms))
            off += 6 * n
    return chunks, off


def _host_arrays(x, weight, bias):
    """Per-core input dicts, all DMA-contiguous, fp16."""
    chunks, total = _sched()
    xp = np.pad(x, ((0, 0), (0, 0), (1, 1), (1, 1)))
    in_maps = []
    for i in range(NCORES):
        slab = xp[:, :, RPC * i:RPC * i + SLAB, :]          # [B, C, 6, 34]
        xs = np.empty((128, XCOLS), dtype=np.float16)
        xs[:, 0:B] = 1.0                                     # ones block
        xr = xs[:, B:].reshape(128, PW, NDR, B)
        for dr in range(NDR):
            pair = slab[:, :, 2 * dr:2 * dr + 2, :]          # [B, C, 2, 34]
            xr[:, :, dr, :] = pair.transpose(2, 1, 3, 0).reshape(128, PW, B)

        w4 = weight[RPC * i:RPC * i + RPC]                   # [4, 32, O, C, 3, 3]
        ws = np.empty((128, total), dtype=np.float16)
        for ch in chunks:
            iw, ows, n, off = ch["iw"], ch["ows"], ch["n"], ch["off"]
            cols = []
            for oh, k0 in ((0, 0), (1, 1), (2, 0), (3, 1)):  # t0..t3 kh pairs
                blocks = [
                    w4[oh, ow, :, :, k0:k0 + 2, iw - ow].transpose(2, 1, 0)
                    .reshape(128, O)
                    for ow in ows
                ]
                cols.append(np.concatenate(blocks, axis=1))
            for top_oh, bot_oh in ((0, 1), (2, 3)):          # t4, t5 singles
                top = np.concatenate(
                    [w4[top_oh, ow, :, :, 2, iw - ow].T for ow in ows], axis=1)
                bot = np.concatenate(
                    [w4[bot_oh, ow, :, :, 0, iw - ow].T for ow in ows], axis=1)
                cols.append(np.concatenate([top, bot], axis=0))
            ws[:, off:off + 6 * n] = np.concatenate(cols, axis=1)

        # bias rows: [1, (q, oh, ow, o)]
        b4 = bias[:, RPC * i:RPC * i + RPC, :].transpose(1, 2, 0)  # [oh, ow, o]
        bse = np.empty((1, NQ * RPC * QO), dtype=np.float16)
        for q in range(NQ):
            bse[0, q * RPC * QO:(q + 1) * RPC * QO] = (
                b4[:, QW * q:QW * q + QW, :].reshape(-1))
        in_maps.append({"xs": np.ascontiguousarray(xs),
                        "ws": np.ascontiguousarray(ws), "bse": bse})
    return in_maps


def _build_program():
    from contextlib import ExitStack
    import concourse.bass as bass
    import concourse.bacc as bacc
    import concourse.tile as tile
    from concourse import mybir

    F16 = mybir.dt.float16
    F32 = mybir.dt.float32
    chunks, total = _sched()

    nc = bacc.Bacc("TRN2", target_bir_lowering=False, debug=False,
                   num_devices=NCORES)
    xs_d = nc.dram_tensor("xs", [128, XCOLS], F16, kind="ExternalInput")
    ws_d = nc.dram_tensor("ws", [128, total], F16, kind="ExternalInput")
    bse_d = nc.dram_tensor("bse", [1, NQ * RPC * QO], F16, kind="ExternalInput")
    out_d = nc.dram_tensor("out", [B, RPC * W * O], F16, kind="ExternalOutput")

    # stop flag on the last MM per (q, oh) bank group
    laststop = set()
    for q in range(NQ):
        seen = {}
        for ci, ch in enumerate(chunks):
            if ch["q"] != q:
                continue
            for mi, mm in enumerate(ch["mms"]):
                seen.setdefault(mm[0], []).append((ci, mi))
        for oh, lst in seen.items():
            laststop.add(lst[-1])

    with ExitStack() as ctx:
        tc = ctx.enter_context(tile.TileContext(nc))
        xpool = ctx.enter_context(tc.tile_pool(name="xs", bufs=1))
        wpool = ctx.enter_context(tc.tile_pool(name="wt", bufs=6))
        bpool = ctx.enter_context(tc.tile_pool(name="bias", bufs=1))
        opool = ctx.enter_context(tc.tile_pool(name="outs", bufs=2))
        pspool = ctx.enter_context(
            tc.tile_pool(name="ps", bufs=2, space=bass.MemorySpace.PSUM))

        NB = RPC * QO  # bias elems per quarter (2048)
        ball = bpool.tile([1, NQ * NB], F16, tag="bias", name="bias_all")

        xq = xpool.tile([128, XCOLS], F16, tag="xq", name="xq")
        ones = xq[0:1, 0:B]

        def xsl(dr, p0, psz, iw):
            base = B + (iw * NDR + dr) * B
            return xq[p0:p0 + psz, base:base + B]

        ws_ap = ws_d.ap()
        xs_ap = xs_d.ap()
        out3 = out_d.ap().rearrange("b (oh r) -> b oh r", r=W * O)
        EVS = 6 * O  # early-evacuation split point within each oh bank (384)
        GROUPS = ((0, 1), (2, 3), (4, 5), (6, 7), (8,), (9,))
        xlo = 0
        for q in range(NQ):
            # x piece for this quarter's new iw range (piece 0 incl. ones)
            xhi = B + XPIECE[q] * NDR * B
            nc.sync.dma_start(xq[:, xlo:xhi], xs_ap[:, xlo:xhi])
            if q == 0:
                nc.sync.dma_start(ball[:], bse_d.ap())
            xlo = xhi

            bt = ball[0:1, q * NB:(q + 1) * NB]
            ps = pspool.tile([B, RPC * QO], F32, tag="psb", name=f"ps{q}")
            for oh in range(RPC):
                nc.tensor.matmul(ps[:, oh * QO:(oh + 1) * QO], ones,
                                 bt[0:1, oh * QO:(oh + 1) * QO],
                                 start=True, stop=False)
            qchunks = [(ci, ch) for ci, ch in enumerate(chunks) if ch["q"] == q]
            ot = opool.tile([B, RPC * QO], F16, tag="ot", name=f"ot{q}")
            ot3 = ot[:].rearrange("b (oh r) -> b oh r", r=QO)
            ps3 = ps[:].rearrange("b (oh r) -> b oh r", r=QO)
            for grp in GROUPS:
                gchunks = [qchunks[j] for j in grp]
                goff = gchunks[0][1]["off"]
                gcols = sum(6 * ch["n"] for _, ch in gchunks)
                wt = wpool.tile([128, gcols], F16, tag="wtile",
                                name=f"wt{q}_{grp[0]}")
                nc.sync.dma_start(wt[:], ws_ap[:, goff:goff + gcols])
                for ci, ch in gchunks:
                    iw, ows, n = ch["iw"], ch["ows"], ch["n"]
                    toff = ch["off"] - goff                  # base col in wt
                    c0 = (ows[0] - QW * q) * O
                    for mi, mm in enumerate(ch["mms"]):
                        oh, dr, p0, psz, ti, tp0 = mm
                        stop = (ci, mi) in laststop
                        xh = xsl(dr, p0, psz, iw)
                        wh = wt[tp0:tp0 + psz, toff + ti * n:toff + ti * n + n]
                        nc.tensor.matmul(ps[:, oh * QO + c0:oh * QO + c0 + n],
                                         xh, wh, start=False, stop=stop)
                if grp[-1] == 7:
                    # cols [0, EVS) of every oh bank are final after chunk 7
                    nc.scalar.copy(ot3[:, :, 0:EVS], ps3[:, :, 0:EVS])
                    nc.scalar.dma_start(
                        out3[:, :, q * QO:q * QO + EVS], ot3[:, :, 0:EVS])
            nc.scalar.copy(ot3[:, :, EVS:QO], ps3[:, :, EVS:QO])
            nc.scalar.dma_start(
                out3[:, :, q * QO + EVS:(q + 1) * QO], ot3[:, :, EVS:QO])

    nc.compile()
    return nc


def kernel(x, weight, bias):
    x = np.asarray(x, dtype=np.float32)
    weight = np.asarray(weight, dtype=np.float32)
    bias = np.asarray(bias, dtype=np.float32)

    from concourse.bass_utils import run_bass_kernel_spmd

    if "nc" not in _cache:
        _cache["nc"] = _build_program()
    nc = _cache["nc"]

    in_maps = _host_arrays(x, weight, bias)
    res = run_bass_kernel_spmd(nc, in_maps, list(range(NCORES)))
    out = np.empty((B, O, H, W), dtype=np.float32)
    for i in range(NCORES):
        o_i = res.results[i]["out"].astype(np.float32)
        o_i = o_i.reshape(B, RPC, W, O)                     # [b, oh_l, ow, o]
        out[:, :, RPC * i:RPC * i + RPC, :] = o_i.transpose(0, 3, 1, 2)
    return out


# revision 26
# speedup vs baseline: 2.1515x; 1.0167x over previous
"""Locally-connected Conv2d (nn.Conv2dLocal) Trainium2 Bass kernel.

Problem (hardcoded):
  x:      [B=64, C=64, H=32, W=32]  f32
  weight: [OH=32, OW=32, O=64, C=64, KH=3, KW=3] f32
  bias:   [O=64, OH=32, OW=32] f32
  out:    [B=64, O=64, OH=32, OW=32] f32
  out[b,o,oh,ow] = bias[o,oh,ow]
      + sum_{c,kh,kw} x[b,c,oh+kh-1,ow+kw-1] * weight[oh,ow,o,c,kh,kw]

Sharding: 8 cores, core i owns output rows oh in [4i, 4i+4).

DMA-minimal fp16 design (~12.2 MB/core, DMA-bound at 360 B/ns):
  - x slab: padded rows 0..5 stored ONCE as 3 even "double rows"
    dr = rows (2dr, 2dr+1), partitions (row parity, c), cols
    (iw, dr, b) iw-major + a leading ones block for the bias matmul.
    [128, 64 + 34*3*64] fp16 = 1.6 MB, DMA'd in 4 iw-range pieces
    interleaved with the weight stream so compute starts early.
  - per output row oh the 3 kh taps split as one full-K pair + one
    half-K single against the even-pair layout:
      oh even: pair (kh0,kh1) = dr oh/2 full;  single kh2 = dr(oh/2+1) top
      oh odd:  single kh0 = dr (oh-1)/2 bottom; pair (kh1,kh2) = dr(oh+1)/2 full
  - weights streamed once, fp16 [128, 36864] = 9.4 MB; per (q, iw)
    chunk 6 tiles: t0..t3 = per-oh kh pairs, t4/t5 = packed singles.
  - bias: one fp16 rank-1 matmul opens each (q, oh) PSUM bank.
  - per quarter one 4-bank PSUM tile [64, 2048]; single wide ScalarE
    cast to fp16 SBUF; out DMA issued from the Activation queue
    (no cross-engine semaphore hop). out = 1.05 MB.
  Rel err ~4e-4 (fp16 rounding of x, w, out; f32 accumulation).
"""

import numpy as np

B, C, H, W = 64, 64, 32, 32
O, KH, KW = 64, 3, 3
NCORES = 8
RPC = 4              # output rows per core
SLAB = RPC + 2       # padded input rows per core
PW = W + 2           # padded width (34)
NDR = SLAB // 2      # even double-rows per slab (3)
NQ = 8               # ow groups (half-quarters)
QW = 4               # ow per group
QCOLS = QW + 2       # iw columns per group (6)
QO = QW * O          # psum cols per (group, oh) (256)
XCOLS = B + PW * NDR * B          # ones block + x cols
XPIECE = (6, 10, 14, 18, 22, 26, 30, PW)  # iw piece upper bounds per group

_cache = {}


def _sched():
    chunks = []
    off = 0
    for q in range(NQ):
        for iw in range(QW * q, QW * q + QCOLS):
            ows = [ow for ow in (iw - 2, iw - 1, iw) if QW * q <= ow < QW * q + QW]
            n = len(ows) * O
            # (oh, dr, p0, psz, tile_idx, tile_p0)
            mms = [
                (0, 0, 0, 128, 0, 0),    # oh0 kh(0,1) = dr0 full
                (1, 1, 0, 128, 1, 0),    # oh1 kh(1,2) = dr1 full
                (2, 1, 0, 128, 2, 0),    # oh2 kh(0,1) = dr1 full
                (3, 2, 0, 128, 3, 0),    # oh3 kh(1,2) = dr2 full
                (0, 1, 0, 64, 4, 0),     # oh0 kh2 = row2 = dr1 top
                (1, 0, 64, 64, 4, 64),   # oh1 kh0 = row1 = dr0 bottom
                (2, 2, 0, 64, 5, 0),     # oh2 kh2 = row4 = dr2 top
                (3, 1, 64, 64, 5, 64),   # oh3 kh0 = row3 = dr1 bottom
            ]
            chunks.append(dict(q=q, iw=iw, ows=ows, n=n, off=off, mms=mms))
            off += 6 * n
    return chunks, off


def _host_arrays(x, weight, bias):
    """Per-core input dicts, all DMA-contiguous, fp16."""
    chunks, total = _sched()
    xp = np.pad(x, ((0, 0), (0, 0), (1, 1), (1, 1)))
    in_maps = []
    for i in range(NCORES):
        slab = xp[:, :, RPC * i:RPC * i + SLAB, :]          # [B, C, 6, 34]
        xs = np.empty((128, XCOLS), dtype=np.float16)
        xs[:, 0:B] = 1.0                                     # ones block
        xr = xs[:, B:].reshape(128, PW, NDR, B)
        for dr in range(NDR):
            pair = slab[:, :, 2 * dr:2 * dr + 2, :]          # [B, C, 2, 34]
            xr[:, :, dr, :] = pair.transpose(2, 1, 3, 0).reshape(128, PW, B)

        w4 = weight[RPC * i:RPC * i + RPC]                   # [4, 32, O, C, 3, 3]
        ws = np.empty((128, total), dtype=np.float16)
        for ch in chunks:
            iw, ows, n, off = ch["iw"], ch["ows"], ch["n"], ch["off"]
            cols = []
            for oh, k0 in ((0, 0), (1, 1), (2, 0), (3, 1)):  # t0..t3 kh pairs
                blocks = [
                    w4[oh, ow, :, :, k0:k0 + 2, iw - ow].transpose(2, 1, 0)
                    .reshape(128, O)
                    for ow in ows
                ]
                cols.append(np.concatenate(blocks, axis=1))
            for top_oh, bot_oh in ((0, 1), (2, 3)):          # t4, t5 singles
                top = np.concatenate(
                    [w4[top_oh, ow, :, :, 2, iw - ow].T for ow in ows], axis=1)
                bot = np.concatenate(
                    [w4[bot_oh, ow, :, :, 0, iw - ow].T for ow in ows], axis=1)
                cols.append(np.concatenate([top, bot], axis=0))
            ws[:, off:off + 6 * n] = np.concatenate(cols, axis=1)

        # bias rows: [1, (q, oh, ow, o)]
        b4 = bias[:, RPC * i:RPC * i + RPC, :].transpose(1, 2, 0)  # [oh, ow, o]
        bse = np.empty((1, NQ * RPC * QO), dtype=np.float16)
        for q in range(NQ):
            bse[0, q * RPC * QO:(q + 1) * RPC * QO] = (
                b4[:, QW * q:QW * q + QW, :].reshape(-1))
        in_maps.append({"xs": np.ascontiguousarray(xs),
                        "ws": np.ascontiguousarray(ws), "bse": bse})
    return in_maps


def _build_program():
    from contextlib import ExitStack
    import concourse.bass as bass
    import concourse.bacc as bacc
    import concourse.tile as tile
    from concourse import mybir

    F16 = mybir.dt.float16
    F32 = mybir.dt.float32
    chunks, total = _sched()

    nc = bacc.Bacc("TRN2", target_bir_lowering=False, debug=False,
                   num_devices=NCORES)
    xs_d = nc.dram_tensor("xs", [128, XCOLS], F16, kind="ExternalInput")
    ws_d = nc.dram_tensor("ws", [128, total], F16, kind="ExternalInput")
    bse_d = nc.dram_tensor("bse", [1, NQ * RPC * QO], F16, kind="ExternalInput")
    out_d = nc.dram_tensor("out", [B, RPC * W * O], F16, kind="ExternalOutput")

    # stop flag on the last MM per (q, oh) bank group
    laststop = set()
    for q in range(NQ):
        seen = {}
        for ci, ch in enumerate(chunks):
            if ch["q"] != q:
                continue
            for mi, mm in enumerate(ch["mms"]):
                seen.setdefault(mm[0], []).append((ci, mi))
        for oh, lst in seen.items():
            laststop.add(lst[-1])

    with ExitStack() as ctx:
        tc = ctx.enter_context(tile.TileContext(nc))
        xpool = ctx.enter_context(tc.tile_pool(name="xs", bufs=1))
        wpool = ctx.enter_context(tc.tile_pool(name="wt", bufs=6))
        bpool = ctx.enter_context(tc.tile_pool(name="bias", bufs=1))
        opool = ctx.enter_context(tc.tile_pool(name="outs", bufs=3))
        pspool = ctx.enter_context(
            tc.tile_pool(name="ps", bufs=2, space=bass.MemorySpace.PSUM))

        NB = RPC * QO  # bias elems per quarter (2048)
        ball = bpool.tile([1, NQ * NB], F16, tag="bias", name="bias_all")

        cpool = ctx.enter_context(tc.tile_pool(name="warm", bufs=1))
        warm = cpool.tile([1, QO], F16, tag="warm", name="warm")
        nc.gpsimd.memset(warm[:], 1.0)
        ones = warm[0:1, 0:B]

        xq = xpool.tile([128, XCOLS], F16, tag="xq", name="xq")

        def xsl(dr, p0, psz, iw):
            base = B + (iw * NDR + dr) * B
            return xq[p0:p0 + psz, base:base + B]

        ws_ap = ws_d.ap()
        xs_ap = xs_d.ap()
        out3 = out_d.ap().rearrange("b (oh r) -> b oh r", r=W * O)
        GROUPS = ((0, 1), (2, 3), (4,), (5,))
        GROUPS_LAST = ((2, 3), (0, 1), (4,), (5,))
        xlo = 0
        for q in range(NQ):
            # x piece for this quarter's new iw range (piece 0 incl. ones)
            xhi = B + XPIECE[q] * NDR * B
            nc.sync.dma_start(xq[:, xlo:xhi], xs_ap[:, xlo:xhi])
            if q == 0:
                nc.sync.dma_start(ball[:], bse_d.ap())
            xlo = xhi

            bt = ball[0:1, q * NB:(q + 1) * NB]
            ps = pspool.tile([B, RPC * 512], F32, tag="psb", name=f"ps{q}")
            # p-state warm-up / boundary filler: keep PE busy while DMAs land
            # so real matmuls run at full clock. Garbage results; bank 0 is
            # reset by the bias matmul's start=True below.
            for wi in range(0):
                nc.tensor.matmul(ps[:, 0:QO], ones, warm[:],
                                 start=True, stop=True, skip_group_check=True)
            for oh in range(RPC):
                nc.tensor.matmul(ps[:, oh * 512:oh * 512 + QO], ones,
                                 bt[0:1, oh * QO:(oh + 1) * QO],
                                 start=True, stop=False)
            qchunks = [(ci, ch) for ci, ch in enumerate(chunks) if ch["q"] == q]
            ot = opool.tile([B, RPC * QO], F16, tag="ot", name=f"ot{q}")
            for grp in GROUPS:
                gchunks = [qchunks[j] for j in grp]
                goff = gchunks[0][1]["off"]
                gcols = sum(6 * ch["n"] for _, ch in gchunks)
                wt = wpool.tile([128, gcols], F16, tag="wtile",
                                name=f"wt{q}_{grp[0]}")
                nc.sync.dma_start(wt[:], ws_ap[:, goff:goff + gcols])
                for ci, ch in gchunks:
                    iw, ows, n = ch["iw"], ch["ows"], ch["n"]
                    toff = ch["off"] - goff                  # base col in wt
                    c0 = (ows[0] - QW * q) * O
                    for mi, mm in enumerate(ch["mms"]):
                        oh, dr, p0, psz, ti, tp0 = mm
                        stop = (ci, mi) in laststop
                        xh = xsl(dr, p0, psz, iw)
                        wh = wt[tp0:tp0 + psz, toff + ti * n:toff + ti * n + n]
                        nc.tensor.matmul(ps[:, oh * 512 + c0:oh * 512 + c0 + n],
                                         xh, wh, start=False, stop=stop)
            ps3 = ps[:].rearrange("b (oh r) -> b oh r", r=512)
            nc.scalar.copy(ot[:].rearrange("b (oh r) -> b oh r", r=QO),
                           ps3[:, :, 0:QO])
            nc.scalar.dma_start(
                out3[:, :, q * QO:(q + 1) * QO],
                ot[:].rearrange("b (oh r) -> b oh r", r=QO))

    nc.compile()
    return nc


def kernel(x, weight, bias):
    x = np.asarray(x, dtype=np.float32)
    weight = np.asarray(weight, dtype=np.float32)
    bias = np.asarray(bias, dtype=np.float32)

    from concourse.bass_utils import run_bass_kernel_spmd

    if "nc" not in _cache:
        _cache["nc"] = _build_program()
    nc = _cache["nc"]

    in_maps = _host_arrays(x, weight, bias)
    res = run_bass_kernel_spmd(nc, in_maps, list(range(NCORES)))
    out = np.empty((B, O, H, W), dtype=np.float32)
    for i in range(NCORES):
        o_i = res.results[i]["out"].astype(np.float32)
        o_i = o_i.reshape(B, RPC, W, O)                     # [b, oh_l, ow, o]
        out[:, :, RPC * i:RPC * i + RPC, :] = o_i.transpose(0, 3, 1, 2)
    return out


# revision 27
# speedup vs baseline: 2.1795x; 1.0130x over previous
"""Locally-connected Conv2d (nn.Conv2dLocal) Trainium2 Bass kernel.

Problem (hardcoded):
  x:      [B=64, C=64, H=32, W=32]  f32
  weight: [OH=32, OW=32, O=64, C=64, KH=3, KW=3] f32
  bias:   [O=64, OH=32, OW=32] f32
  out:    [B=64, O=64, OH=32, OW=32] f32
  out[b,o,oh,ow] = bias[o,oh,ow]
      + sum_{c,kh,kw} x[b,c,oh+kh-1,ow+kw-1] * weight[oh,ow,o,c,kh,kw]

Sharding: 8 cores, core i owns output rows oh in [4i, 4i+4).

DMA-minimal fp16 design (~12.2 MB/core, DMA-bound at 360 B/ns):
  - x slab: padded rows 0..5 stored ONCE as 3 even "double rows"
    dr = rows (2dr, 2dr+1), partitions (row parity, c), cols
    (iw, dr, b) iw-major + a leading ones block for the bias matmul.
    [128, 64 + 34*3*64] fp16 = 1.6 MB, DMA'd in 4 iw-range pieces
    interleaved with the weight stream so compute starts early.
  - per output row oh the 3 kh taps split as one full-K pair + one
    half-K single against the even-pair layout:
      oh even: pair (kh0,kh1) = dr oh/2 full;  single kh2 = dr(oh/2+1) top
      oh odd:  single kh0 = dr (oh-1)/2 bottom; pair (kh1,kh2) = dr(oh+1)/2 full
  - weights streamed once, fp16 [128, 36864] = 9.4 MB; per (q, iw)
    chunk 6 tiles: t0..t3 = per-oh kh pairs, t4/t5 = packed singles.
  - bias: one fp16 rank-1 matmul opens each (q, oh) PSUM bank.
  - per quarter one 4-bank PSUM tile [64, 2048]; single wide ScalarE
    cast to fp16 SBUF; out DMA issued from the Activation queue
    (no cross-engine semaphore hop). out = 1.05 MB.
  Rel err ~4e-4 (fp16 rounding of x, w, out; f32 accumulation).
"""

import numpy as np

B, C, H, W = 64, 64, 32, 32
O, KH, KW = 64, 3, 3
NCORES = 8
RPC = 4              # output rows per core
SLAB = RPC + 2       # padded input rows per core
PW = W + 2           # padded width (34)
NDR = SLAB // 2      # even double-rows per slab (3)
NQ = 8               # ow groups (half-quarters)
QW = 4               # ow per group
QCOLS = QW + 2       # iw columns per group (6)
QO = QW * O          # psum cols per (group, oh) (256)
XCOLS = B + PW * NDR * B          # ones block + x cols
XPIECE = (6, 10, 14, 18, 22, 26, 30, PW)  # iw piece upper bounds per group

_cache = {}


def _sched():
    chunks = []
    off = 0
    for q in range(NQ):
        for iw in range(QW * q, QW * q + QCOLS):
            ows = [ow for ow in (iw - 2, iw - 1, iw) if QW * q <= ow < QW * q + QW]
            n = len(ows) * O
            # (oh, dr, p0, psz, tile_idx, tile_p0)
            mms = [
                (0, 0, 0, 128, 0, 0),    # oh0 kh(0,1) = dr0 full
                (1, 1, 0, 128, 1, 0),    # oh1 kh(1,2) = dr1 full
                (2, 1, 0, 128, 2, 0),    # oh2 kh(0,1) = dr1 full
                (3, 2, 0, 128, 3, 0),    # oh3 kh(1,2) = dr2 full
                (0, 1, 0, 64, 4, 0),     # oh0 kh2 = row2 = dr1 top
                (1, 0, 64, 64, 4, 64),   # oh1 kh0 = row1 = dr0 bottom
                (2, 2, 0, 64, 5, 0),     # oh2 kh2 = row4 = dr2 top
                (3, 1, 64, 64, 5, 64),   # oh3 kh0 = row3 = dr1 bottom
            ]
            chunks.append(dict(q=q, iw=iw, ows=ows, n=n, off=off, mms=mms))
            off += 6 * n
    return chunks, off


def _host_arrays(x, weight, bias):
    """Per-core input dicts, all DMA-contiguous, fp16."""
    chunks, total = _sched()
    xp = np.pad(x, ((0, 0), (0, 0), (1, 1), (1, 1)))
    in_maps = []
    for i in range(NCORES):
        slab = xp[:, :, RPC * i:RPC * i + SLAB, :]          # [B, C, 6, 34]
        xs = np.empty((128, XCOLS), dtype=np.float16)
        xs[:, 0:B] = 1.0                                     # ones block
        xr = xs[:, B:].reshape(128, PW, NDR, B)
        for dr in range(NDR):
            pair = slab[:, :, 2 * dr:2 * dr + 2, :]          # [B, C, 2, 34]
            xr[:, :, dr, :] = pair.transpose(2, 1, 3, 0).reshape(128, PW, B)

        w4 = weight[RPC * i:RPC * i + RPC]                   # [4, 32, O, C, 3, 3]
        ws = np.empty((128, total), dtype=np.float16)
        for ch in chunks:
            iw, ows, n, off = ch["iw"], ch["ows"], ch["n"], ch["off"]
            cols = []
            for oh, k0 in ((0, 0), (1, 1), (2, 0), (3, 1)):  # t0..t3 kh pairs
                blocks = [
                    w4[oh, ow, :, :, k0:k0 + 2, iw - ow].transpose(2, 1, 0)
                    .reshape(128, O)
                    for ow in ows
                ]
                cols.append(np.concatenate(blocks, axis=1))
            for top_oh, bot_oh in ((0, 1), (2, 3)):          # t4, t5 singles
                top = np.concatenate(
                    [w4[top_oh, ow, :, :, 2, iw - ow].T for ow in ows], axis=1)
                bot = np.concatenate(
                    [w4[bot_oh, ow, :, :, 0, iw - ow].T for ow in ows], axis=1)
                cols.append(np.concatenate([top, bot], axis=0))
            ws[:, off:off + 6 * n] = np.concatenate(cols, axis=1)

        # bias rows: [1, (q, oh, ow, o)]
        b4 = bias[:, RPC * i:RPC * i + RPC, :].transpose(1, 2, 0)  # [oh, ow, o]
        bse = np.empty((1, NQ * RPC * QO), dtype=np.float16)
        for q in range(NQ):
            bse[0, q * RPC * QO:(q + 1) * RPC * QO] = (
                b4[:, QW * q:QW * q + QW, :].reshape(-1))
        in_maps.append({"xs": np.ascontiguousarray(xs),
                        "ws": np.ascontiguousarray(ws), "bse": bse})
    return in_maps


def _build_program():
    from contextlib import ExitStack
    import concourse.bass as bass
    import concourse.bacc as bacc
    import concourse.tile as tile
    from concourse import mybir

    F16 = mybir.dt.float16
    F32 = mybir.dt.float32
    chunks, total = _sched()

    nc = bacc.Bacc("TRN2", target_bir_lowering=False, debug=False,
                   num_devices=NCORES)
    xs_d = nc.dram_tensor("xs", [128, XCOLS], F16, kind="ExternalInput")
    ws_d = nc.dram_tensor("ws", [128, total], F16, kind="ExternalInput")
    bse_d = nc.dram_tensor("bse", [1, NQ * RPC * QO], F16, kind="ExternalInput")
    out_d = nc.dram_tensor("out", [B, RPC * W * O], F16, kind="ExternalOutput")

    # stop flag on the last MM per (q, oh) bank group
    laststop = set()
    for q in range(NQ):
        seen = {}
        for ci, ch in enumerate(chunks):
            if ch["q"] != q:
                continue
            for mi, mm in enumerate(ch["mms"]):
                seen.setdefault(mm[0], []).append((ci, mi))
        for oh, lst in seen.items():
            laststop.add(lst[-1])

    with ExitStack() as ctx:
        tc = ctx.enter_context(tile.TileContext(nc))
        xpool = ctx.enter_context(tc.tile_pool(name="xs", bufs=1))
        wpool = ctx.enter_context(tc.tile_pool(name="wt", bufs=6))
        bpool = ctx.enter_context(tc.tile_pool(name="bias", bufs=1))
        opool = ctx.enter_context(tc.tile_pool(name="outs", bufs=3))
        pspool = ctx.enter_context(
            tc.tile_pool(name="ps", bufs=2, space=bass.MemorySpace.PSUM))

        NB = RPC * QO  # bias elems per quarter (2048)
        ball = bpool.tile([1, NQ * NB], F16, tag="bias", name="bias_all")

        cpool = ctx.enter_context(tc.tile_pool(name="warm", bufs=1))
        warm = cpool.tile([1, QO], F16, tag="warm", name="warm")
        nc.gpsimd.memset(warm[:], 1.0)
        ones = warm[0:1, 0:B]

        xq = xpool.tile([128, XCOLS], F16, tag="xq", name="xq")

        def xsl(dr, p0, psz, iw):
            base = B + (iw * NDR + dr) * B
            return xq[p0:p0 + psz, base:base + B]

        ws_ap = ws_d.ap()
        xs_ap = xs_d.ap()
        out3 = out_d.ap().rearrange("b (oh r) -> b oh r", r=W * O)
        GROUPS = ((0, 1), (2, 3), (4,), (5,))
        GROUPS_LAST = ((2, 3), (0, 1), (4,), (5,))
        xlo = 0
        for q in range(NQ):
            # x piece for this quarter's new iw range (piece 0 incl. ones)
            xhi = B + XPIECE[q] * NDR * B
            nc.sync.dma_start(xq[:, xlo:xhi], xs_ap[:, xlo:xhi])
            if q == 0:
                nc.sync.dma_start(ball[:], bse_d.ap())
            xlo = xhi

            bt = ball[0:1, q * NB:(q + 1) * NB]
            ps = pspool.tile([B, RPC * 512], F32, tag="psb", name=f"ps{q}")
            # p-state warm-up / boundary filler: keep PE busy while DMAs land
            # so real matmuls run at full clock. Garbage results; bank 0 is
            # reset by the bias matmul's start=True below.
            for wi in range(0):
                nc.tensor.matmul(ps[:, 0:QO], ones, warm[:],
                                 start=True, stop=True, skip_group_check=True)
            for oh in range(RPC):
                nc.tensor.matmul(ps[:, oh * 512:oh * 512 + QO], ones,
                                 bt[0:1, oh * QO:(oh + 1) * QO],
                                 start=True, stop=False)
            qchunks = [(ci, ch) for ci, ch in enumerate(chunks) if ch["q"] == q]
            ot = opool.tile([B, RPC * QO], F16, tag="ot", name=f"ot{q}")
            for grp in (GROUPS_LAST if q == NQ - 1 else GROUPS):
                gchunks = [qchunks[j] for j in grp]
                goff = gchunks[0][1]["off"]
                gcols = sum(6 * ch["n"] for _, ch in gchunks)
                wt = wpool.tile([128, gcols], F16, tag="wtile",
                                name=f"wt{q}_{grp[0]}")
                nc.sync.dma_start(wt[:], ws_ap[:, goff:goff + gcols])
                for ci, ch in gchunks:
                    iw, ows, n = ch["iw"], ch["ows"], ch["n"]
                    toff = ch["off"] - goff                  # base col in wt
                    c0 = (ows[0] - QW * q) * O
                    for mi, mm in enumerate(ch["mms"]):
                        oh, dr, p0, psz, ti, tp0 = mm
                        stop = (ci, mi) in laststop
                        xh = xsl(dr, p0, psz, iw)
                        wh = wt[tp0:tp0 + psz, toff + ti * n:toff + ti * n + n]
                        nc.tensor.matmul(ps[:, oh * 512 + c0:oh * 512 + c0 + n],
                                         xh, wh, start=False, stop=stop)
            ps3 = ps[:].rearrange("b (oh r) -> b oh r", r=512)
            nc.scalar.copy(ot[:].rearrange("b (oh r) -> b oh r", r=QO),
                           ps3[:, :, 0:QO])
            nc.scalar.dma_start(
                out3[:, :, q * QO:(q + 1) * QO],
                ot[:].rearrange("b (oh r) -> b oh r", r=QO))

    nc.compile()
    return nc


def kernel(x, weight, bias):
    x = np.asarray(x, dtype=np.float32)
    weight = np.asarray(weight, dtype=np.float32)
    bias = np.asarray(bias, dtype=np.float32)

    from concourse.bass_utils import run_bass_kernel_spmd

    if "nc" not in _cache:
        _cache["nc"] = _build_program()
    nc = _cache["nc"]

    in_maps = _host_arrays(x, weight, bias)
    res = run_bass_kernel_spmd(nc, in_maps, list(range(NCORES)))
    out = np.empty((B, O, H, W), dtype=np.float32)
    for i in range(NCORES):
        o_i = res.results[i]["out"].astype(np.float32)
        o_i = o_i.reshape(B, RPC, W, O)                     # [b, oh_l, ow, o]
        out[:, :, RPC * i:RPC * i + RPC, :] = o_i.transpose(0, 3, 1, 2)
    return out


# revision 29
# speedup vs baseline: 2.1804x; 1.0004x over previous
"""Locally-connected Conv2d (nn.Conv2dLocal) Trainium2 Bass kernel.

Problem (hardcoded):
  x:      [B=64, C=64, H=32, W=32]  f32
  weight: [OH=32, OW=32, O=64, C=64, KH=3, KW=3] f32
  bias:   [O=64, OH=32, OW=32] f32
  out:    [B=64, O=64, OH=32, OW=32] f32
  out[b,o,oh,ow] = bias[o,oh,ow]
      + sum_{c,kh,kw} x[b,c,oh+kh-1,ow+kw-1] * weight[oh,ow,o,c,kh,kw]

Sharding: 8 cores, core i owns output rows oh in [4i, 4i+4).

DMA-minimal fp16 design (~12.2 MB/core, DMA-bound at 360 B/ns):
  - x slab: padded rows 0..5 stored ONCE as 3 even "double rows"
    dr = rows (2dr, 2dr+1), partitions (row parity, c), cols
    (iw, dr, b) iw-major + a leading ones block for the bias matmul.
    [128, 64 + 34*3*64] fp16 = 1.6 MB, DMA'd in 4 iw-range pieces
    interleaved with the weight stream so compute starts early.
  - per output row oh the 3 kh taps split as one full-K pair + one
    half-K single against the even-pair layout:
      oh even: pair (kh0,kh1) = dr oh/2 full;  single kh2 = dr(oh/2+1) top
      oh odd:  single kh0 = dr (oh-1)/2 bottom; pair (kh1,kh2) = dr(oh+1)/2 full
  - weights streamed once, fp16 [128, 36864] = 9.4 MB; per (q, iw)
    chunk 6 tiles: t0..t3 = per-oh kh pairs, t4/t5 = packed singles.
  - bias: one fp16 rank-1 matmul opens each (q, oh) PSUM bank.
  - per quarter one 4-bank PSUM tile [64, 2048]; single wide ScalarE
    cast to fp16 SBUF; out DMA issued from the Activation queue
    (no cross-engine semaphore hop). out = 1.05 MB.
  Rel err ~4e-4 (fp16 rounding of x, w, out; f32 accumulation).
"""

import numpy as np

B, C, H, W = 64, 64, 32, 32
O, KH, KW = 64, 3, 3
NCORES = 8
RPC = 4              # output rows per core
SLAB = RPC + 2       # padded input rows per core
PW = W + 2           # padded width (34)
NDR = SLAB // 2      # even double-rows per slab (3)
NQ = 8               # ow groups (half-quarters)
QW = 4               # ow per group
QCOLS = QW + 2       # iw columns per group (6)
QO = QW * O          # psum cols per (group, oh) (256)
XCOLS = B + PW * NDR * B          # ones block + x cols
XPIECE = (6, 10, 14, 18, 22, 26, 30, PW)  # iw piece upper bounds per group

_cache = {}


def _sched():
    chunks = []
    off = 0
    for q in range(NQ):
        for iw in range(QW * q, QW * q + QCOLS):
            ows = [ow for ow in (iw - 2, iw - 1, iw) if QW * q <= ow < QW * q + QW]
            n = len(ows) * O
            # (oh, dr, p0, psz, tile_idx, tile_p0)
            mms = [
                (0, 0, 0, 128, 0, 0),    # oh0 kh(0,1) = dr0 full
                (1, 1, 0, 128, 1, 0),    # oh1 kh(1,2) = dr1 full
                (2, 1, 0, 128, 2, 0),    # oh2 kh(0,1) = dr1 full
                (3, 2, 0, 128, 3, 0),    # oh3 kh(1,2) = dr2 full
                (0, 1, 0, 64, 4, 0),     # oh0 kh2 = row2 = dr1 top
                (1, 0, 64, 64, 4, 64),   # oh1 kh0 = row1 = dr0 bottom
                (2, 2, 0, 64, 5, 0),     # oh2 kh2 = row4 = dr2 top
                (3, 1, 64, 64, 5, 64),   # oh3 kh0 = row3 = dr1 bottom
            ]
            chunks.append(dict(q=q, iw=iw, ows=ows, n=n, off=off, mms=mms))
            off += 6 * n
    return chunks, off


def _host_arrays(x, weight, bias):
    """Per-core input dicts, all DMA-contiguous, fp16."""
    chunks, total = _sched()
    xp = np.pad(x, ((0, 0), (0, 0), (1, 1), (1, 1)))
    in_maps = []
    for i in range(NCORES):
        slab = xp[:, :, RPC * i:RPC * i + SLAB, :]          # [B, C, 6, 34]
        xs = np.empty((128, XCOLS), dtype=np.float16)
        xs[:, 0:B] = 1.0                                     # ones block
        xr = xs[:, B:].reshape(128, PW, NDR, B)
        for dr in range(NDR):
            pair = slab[:, :, 2 * dr:2 * dr + 2, :]          # [B, C, 2, 34]
            xr[:, :, dr, :] = pair.transpose(2, 1, 3, 0).reshape(128, PW, B)

        w4 = weight[RPC * i:RPC * i + RPC]                   # [4, 32, O, C, 3, 3]
        ws = np.empty((128, total), dtype=np.float16)
        for ch in chunks:
            iw, ows, n, off = ch["iw"], ch["ows"], ch["n"], ch["off"]
            cols = []
            for oh, k0 in ((0, 0), (1, 1), (2, 0), (3, 1)):  # t0..t3 kh pairs
                blocks = [
                    w4[oh, ow, :, :, k0:k0 + 2, iw - ow].transpose(2, 1, 0)
                    .reshape(128, O)
                    for ow in ows
                ]
                cols.append(np.concatenate(blocks, axis=1))
            for top_oh, bot_oh in ((0, 1), (2, 3)):          # t4, t5 singles
                top = np.concatenate(
                    [w4[top_oh, ow, :, :, 2, iw - ow].T for ow in ows], axis=1)
                bot = np.concatenate(
                    [w4[bot_oh, ow, :, :, 0, iw - ow].T for ow in ows], axis=1)
                cols.append(np.concatenate([top, bot], axis=0))
            ws[:, off:off + 6 * n] = np.concatenate(cols, axis=1)

        # bias rows: [1, (q, oh, ow, o)]
        b4 = bias[:, RPC * i:RPC * i + RPC, :].transpose(1, 2, 0)  # [oh, ow, o]
        bse = np.empty((1, NQ * RPC * QO), dtype=np.float16)
        for q in range(NQ):
            bse[0, q * RPC * QO:(q + 1) * RPC * QO] = (
                b4[:, QW * q:QW * q + QW, :].reshape(-1))
        in_maps.append({"xs": np.ascontiguousarray(xs),
                        "ws": np.ascontiguousarray(ws), "bse": bse})
    return in_maps


def _build_program():
    from contextlib import ExitStack
    import concourse.bass as bass
    import concourse.bacc as bacc
    import concourse.tile as tile
    from concourse import mybir

    F16 = mybir.dt.float16
    F32 = mybir.dt.float32
    chunks, total = _sched()

    nc = bacc.Bacc("TRN2", target_bir_lowering=False, debug=False,
                   num_devices=NCORES)
    xs_d = nc.dram_tensor("xs", [128, XCOLS], F16, kind="ExternalInput")
    ws_d = nc.dram_tensor("ws", [128, total], F16, kind="ExternalInput")
    bse_d = nc.dram_tensor("bse", [1, NQ * RPC * QO], F16, kind="ExternalInput")
    out_d = nc.dram_tensor("out", [B, RPC * W * O], F16, kind="ExternalOutput")

    # stop flag on the last MM per (q, oh) bank group
    laststop = set()
    for q in range(NQ):
        seen = {}
        for ci, ch in enumerate(chunks):
            if ch["q"] != q:
                continue
            for mi, mm in enumerate(ch["mms"]):
                seen.setdefault(mm[0], []).append((ci, mi))
        for oh, lst in seen.items():
            laststop.add(lst[-1])

    with ExitStack() as ctx:
        tc = ctx.enter_context(tile.TileContext(nc))
        xpool = ctx.enter_context(tc.tile_pool(name="xs", bufs=1))
        wpool = ctx.enter_context(tc.tile_pool(name="wt", bufs=6))
        bpool = ctx.enter_context(tc.tile_pool(name="bias", bufs=1))
        opool = ctx.enter_context(tc.tile_pool(name="outs", bufs=3))
        pspool = ctx.enter_context(
            tc.tile_pool(name="ps", bufs=2, space=bass.MemorySpace.PSUM))

        NB = RPC * QO  # bias elems per quarter (2048)
        ball = bpool.tile([1, NQ * NB], F16, tag="bias", name="bias_all")

        cpool = ctx.enter_context(tc.tile_pool(name="warm", bufs=1))
        warm = cpool.tile([1, QO], F16, tag="warm", name="warm")
        nc.gpsimd.memset(warm[:], 1.0)
        ones = warm[0:1, 0:B]

        xq = xpool.tile([128, XCOLS], F16, tag="xq", name="xq")

        def xsl(dr, p0, psz, iw):
            base = B + (iw * NDR + dr) * B
            return xq[p0:p0 + psz, base:base + B]

        ws_ap = ws_d.ap()
        xs_ap = xs_d.ap()
        out3 = out_d.ap().rearrange("b (oh r) -> b oh r", r=W * O)
        GROUPS = ((0, 1), (2, 3), (4,), (5,))
        GROUPS_LAST = ((2, 3), (0, 1), (4, 5))
        xlo = 0
        for q in range(NQ):
            # x piece for this quarter's new iw range (piece 0 incl. ones)
            xhi = B + XPIECE[q] * NDR * B
            nc.sync.dma_start(xq[:, xlo:xhi], xs_ap[:, xlo:xhi])
            if q == 0:
                nc.sync.dma_start(ball[:], bse_d.ap())
            xlo = xhi

            bt = ball[0:1, q * NB:(q + 1) * NB]
            ps = pspool.tile([B, RPC * 512], F32, tag="psb", name=f"ps{q}")
            # p-state warm-up / boundary filler: keep PE busy while DMAs land
            # so real matmuls run at full clock. Garbage results; bank 0 is
            # reset by the bias matmul's start=True below.
            for wi in range(0):
                nc.tensor.matmul(ps[:, 0:QO], ones, warm[:],
                                 start=True, stop=True, skip_group_check=True)
            for oh in range(RPC):
                nc.tensor.matmul(ps[:, oh * 512:oh * 512 + QO], ones,
                                 bt[0:1, oh * QO:(oh + 1) * QO],
                                 start=True, stop=False)
            qchunks = [(ci, ch) for ci, ch in enumerate(chunks) if ch["q"] == q]
            ot = opool.tile([B, RPC * QO], F16, tag="ot", name=f"ot{q}")
            for grp in (GROUPS_LAST if q == NQ - 1 else GROUPS):
                gchunks = [qchunks[j] for j in grp]
                goff = gchunks[0][1]["off"]
                gcols = sum(6 * ch["n"] for _, ch in gchunks)
                wt = wpool.tile([128, gcols], F16, tag="wtile",
                                name=f"wt{q}_{grp[0]}")
                nc.sync.dma_start(wt[:], ws_ap[:, goff:goff + gcols])
                for ci, ch in gchunks:
                    iw, ows, n = ch["iw"], ch["ows"], ch["n"]
                    toff = ch["off"] - goff                  # base col in wt
                    c0 = (ows[0] - QW * q) * O
                    for mi, mm in enumerate(ch["mms"]):
                        oh, dr, p0, psz, ti, tp0 = mm
                        stop = (ci, mi) in laststop
                        xh = xsl(dr, p0, psz, iw)
                        wh = wt[tp0:tp0 + psz, toff + ti * n:toff + ti * n + n]
                        nc.tensor.matmul(ps[:, oh * 512 + c0:oh * 512 + c0 + n],
                                         xh, wh, start=False, stop=stop)
            ps3 = ps[:].rearrange("b (oh r) -> b oh r", r=512)
            nc.scalar.copy(ot[:].rearrange("b (oh r) -> b oh r", r=QO),
                           ps3[:, :, 0:QO])
            nc.scalar.dma_start(
                out3[:, :, q * QO:(q + 1) * QO],
                ot[:].rearrange("b (oh r) -> b oh r", r=QO))

    nc.compile()
    return nc


def kernel(x, weight, bias):
    x = np.asarray(x, dtype=np.float32)
    weight = np.asarray(weight, dtype=np.float32)
    bias = np.asarray(bias, dtype=np.float32)

    from concourse.bass_utils import run_bass_kernel_spmd

    if "nc" not in _cache:
        _cache["nc"] = _build_program()
    nc = _cache["nc"]

    in_maps = _host_arrays(x, weight, bias)
    res = run_bass_kernel_spmd(nc, in_maps, list(range(NCORES)))
    out = np.empty((B, O, H, W), dtype=np.float32)
    for i in range(NCORES):
        o_i = res.results[i]["out"].astype(np.float32)
        o_i = o_i.reshape(B, RPC, W, O)                     # [b, oh_l, ow, o]
        out[:, :, RPC * i:RPC * i + RPC, :] = o_i.transpose(0, 3, 1, 2)
    return out
